# revision 23
# baseline (speedup 1.0000x reference)
"""Trainium2 Bass kernel for the DDF (dynamic-filter + ECA + BN) module.

Data-parallel over batch B=8 across 8 NeuronCores (one image per core),
params replicated, sync-BN via a single small AllReduce.

Layout: channels on partitions (CT=2 channel-tiles of 128); x lives in ONE
SBUF buffer with a 65-elem row stride whose 65th column is a zero "gutter".
All nine 3x3 window shifts are then plain slices of that buffer (the gutter
supplies the zero-pad at the row edges), so no derived shifted copies, no
wrap fix-ups.

Per 16-row chunk, the 18 mm1 PSUM tiles are drained by three engines in
parallel so the PE never waits on a PSUM bank:
  - Scalar taps: ACT evict (+bias) to SBUF, DVE does the bf16 window product.
  - Pool taps:   gpsimd scalar_tensor_tensor does (psum+bias)*window in one op.
The 9 tap products are summed by an 8-add DVE tree ordered by readiness, and
mm2 contracts {fused, attn*x} through W_proj (x-branch via attention-scaled
weights).  mm2 of chunk i is interleaved into mm1 of chunk i+1.  BN stats are
taken from the mm2 PSUM tiles (DVE bn_stats), aggregated, and exchanged with
one 2KB AllReduce on the sync queue; the Sqrt act-table load is hidden under
the AllReduce.  y is kept in bf16 and normalized in place, then written out
on two DMA queues.
"""

import os

import numpy as np
import ml_dtypes

import concourse.bass as bass
import concourse.mybir as mybir
import concourse.tile as tile
from concourse import bacc
from concourse.bass_utils import run_bass_kernel_spmd

B, C, H, W = 8, 256, 64, 64
KS = 3
HW = H * W                    # 4096
SW = W + 1                    # row stride with zero gutter column
G = SW + 1                    # guard elems at each end (covers di,dj = -1,-1)
XB = G + H * SW + G           # 4292 per channel-tile
NCORES = 8
CT = 2                        # channel tiles of 128
MT1 = KS * KS * CT            # 18 mm1 output m-tiles
BN_EPS = 1e-5
F32 = mybir.dt.float32
BF16 = mybir.dt.bfloat16
ROWS = 16                     # rows per chunk
NCHUNKS = H // ROWS           # 4
CHUNK = ROWS * W              # 1024 pixels per chunk per channel-tile
NH = CHUNK // 512             # 512-px matmul groups per chunk

AF = mybir.ActivationFunctionType
ALU = mybir.AluOpType
RG = [list(range(NCORES))]

# Pool cannot read PSUM on TRN2 (and only supports tensor_tensor, not stt),
# so the 18 mm1 PSUM tiles per chunk drain through Scalar (ACT evict+bias)
# and DVE (fused stt) only; Pool multiplies scalar-evicted taps against
# their windows from SBUF.
POOL_TAPS = (3, 5, 6, 8)      # scalar evict + gpsimd tensor_tensor product
DVE_STT_TAPS = (2, 7)         # DVE (psum+bias)*window in one op
# remaining taps (0, 1, 4): scalar evict + DVE bf16 product


def _emit(tc):
    nc = tc.nc

    xgp = nc.declare_dram_parameter("xg", [CT, 128, XB], BF16, isOutput=False)
    wf = nc.declare_dram_parameter("wf", [CT, 128, MT1 * 128], BF16, isOutput=False)
    # misc fp32 params packed: bfp[18] | weca[3] | gam[2] | bet[2]
    misc = nc.declare_dram_parameter("misc", [128, MT1 + 7], F32, isOutput=False)
    wp = nc.declare_dram_parameter("wp", [128, CT * C], BF16, isOutput=False)
    yout = nc.declare_dram_parameter("y", [CT, 128, HW], BF16, isOutput=True)

    with (
        tc.tile_pool(name="consts", bufs=1) as consts,
        tc.tile_pool(name="fps", bufs=3, space="PSUM") as fps,
        tc.tile_pool(name="yps", bufs=2, space="PSUM") as yps,
        tc.tile_pool(name="fsb", bufs=5) as fsb_pool,
        tc.tile_pool(name="prod", bufs=1) as prod_pool,
        tc.tile_pool(name="dram", bufs=1, space="DRAM") as dram,
    ):
        # ---- resident tensors -------------------------------------------
        wf_sb = [consts.tile([128, MT1 * 128], BF16, tag=f"wf{kt}", name=f"wf{kt}")
                 for kt in range(CT)]
        wpb = consts.tile([128, CT, C], BF16, tag="wpb", name="wpb")
        wp_sb = [wpb[:, kt, :] for kt in range(CT)]
        weffb = consts.tile([128, CT, C], BF16, tag="weffb", name="weffb")
        weff = [weffb[:, kt, :] for kt in range(CT)]
        miscb = consts.tile([128, MT1 + 7], F32, tag="miscb", name="miscb")
        bfp_sb = miscb[:, 0:MT1]
        wecab = miscb[:, MT1 : MT1 + 3]
        gam_sb = miscb[:, MT1 + 3 : MT1 + 5]
        bet_sb = miscb[:, MT1 + 5 : MT1 + 7]
        xg = consts.tile([128, CT, XB], BF16, tag="xg", name="xg")
        y_sb = [consts.tile([128, HW], BF16, tag=f"ysb{mt}", name=f"ysb{mt}")
                for mt in range(CT)]
        stats_sb = [
            consts.tile([128, NCHUNKS * NH, 6], F32, tag=f"st{mt}", name=f"st{mt}")
            for mt in range(CT)
        ]
        pscr = consts.tile([128, ROWS * SW], F32, tag="pscr", name="pscr")
        pacc = consts.tile([128, CT, NCHUNKS], F32, tag="pacc", name="pacc")
        zb = consts.tile([128, 1], F32, tag="zb", name="zb")
        nc.vector.memset(zb[:], 0.0)

        # ---- collective warmup ------------------------------------------
        warm_in = dram.tile([128, 1], F32, tag="wi", name="wi")
        warm_out = dram.tile([128, 1], F32, tag="wo", name="wo",
                             addr_space="Shared")
        nc.sync.dma_start(out=warm_in[:], in_=zb[:])
        nc.gpsimd.collective_compute(
            "AllReduce", ALU.add, replica_groups=RG,
            ins=[warm_in[:].opt()], outs=[warm_out[:].opt()],
        )

        # ---- input DMAs --------------------------------------------------
        # x in 4 overlapping row pieces per ct (halo rows included so chunk
        # i's windows only read pieces <= i); ct0 on sync, ct1 on gpsimd.
        for i in range(NCHUNKS):
            lo = 0 if i == 0 else G + (ROWS * i - 1) * SW - 2
            hi = XB if i == NCHUNKS - 1 else G + (ROWS * i + ROWS + 1) * SW
            nc.sync.dma_start(out=xg[:, 0, lo:hi], in_=xgp[0, :, lo:hi])
            nc.gpsimd.dma_start(out=xg[:, 1, lo:hi], in_=xgp[1, :, lo:hi])
        for kt in range(CT):
            nc.scalar.dma_start(out=wf_sb[kt][:], in_=wf[kt])
        nc.scalar.dma_start(out=miscb[:], in_=misc[:, :])
        nc.scalar.dma_start(
            out=wpb.rearrange("p c x -> p (c x)"), in_=wp[:, :]
        )

        # ---- window / center access patterns ----------------------------
        def win_all(ci, k):
            di, dj = divmod(k, KS)
            off = G + (ROWS * ci + di - 1) * SW + (dj - 1)
            return xg[:, :, off : off + ROWS * SW].rearrange(
                "p c (r w) -> p c r w", w=SW)[:, :, :, 0:W]

        def win_ct(ci, k, ct):
            di, dj = divmod(k, KS)
            off = G + (ROWS * ci + di - 1) * SW + (dj - 1)
            return xg[:, ct, off : off + ROWS * SW].rearrange(
                "p (r w) -> p r w", w=SW)[:, :, 0:W]

        def center(ci, kt, nh):
            off = G + (ROWS * ci + 8 * nh) * SW
            return xg[:, kt, off : off + 8 * SW].rearrange(
                "p (r w) -> p r w", w=SW)[:, :, 0:W]

        # ---- ECA pooling -------------------------------------------------
        # pieces 0,1 on DVE (head slack), 2,3 on scalar accum (hooked)
        def pool_dve(ci):
            lo = G + ROWS * ci * SW
            nc.vector.tensor_reduce(
                out=pacc[:, :, ci : ci + 1],
                in_=xg[:, :, lo : lo + ROWS * SW],
                axis=mybir.AxisListType.X,
                op=ALU.add,
            )

        def pool_scalar(ci):
            lo = G + ROWS * ci * SW
            for ct in range(CT):
                nc.scalar.activation(
                    out=pscr[:], in_=xg[:, ct, lo : lo + ROWS * SW],
                    func=AF.Copy, accum_out=pacc[:, ct, ci : ci + 1],
                )



        pool2 = consts.tile([128, CT], F32, tag="pool2", name="pool2")
        shd = consts.tile([128, CT], F32, tag="shd", name="shd")
        shu = consts.tile([128, CT], F32, tag="shu", name="shu")
        eca1 = consts.tile([128, CT], F32, tag="eca1", name="eca1")
        eca2 = consts.tile([128, CT], F32, tag="eca2", name="eca2")
        attn = consts.tile([128, CT], F32, tag="attn", name="attn")

        def emit_eca_combine():
            nc.gpsimd.tensor_tensor(
                out=pool2[:], in0=pacc[:, :, 0], in1=pacc[:, :, 1], op=ALU.add
            )
            nc.gpsimd.tensor_tensor(
                out=pool2[:], in0=pool2[:], in1=pacc[:, :, 2], op=ALU.add
            )
            nc.gpsimd.tensor_tensor(
                out=pool2[:], in0=pool2[:], in1=pacc[:, :, 3], op=ALU.add
            )
            nc.gpsimd.memset(shd[:], 0.0)
            nc.gpsimd.memset(shu[:], 0.0)
            for ct in range(CT):
                nc.gpsimd.dma_start(
                    out=shd[1:128, ct : ct + 1], in_=pool2[0:127, ct : ct + 1]
                )
                nc.gpsimd.dma_start(
                    out=shu[0:127, ct : ct + 1], in_=pool2[1:128, ct : ct + 1]
                )
            nc.gpsimd.dma_start(out=shd[0:1, 1:2], in_=pool2[127:128, 0:1])
            nc.gpsimd.dma_start(out=shu[127:128, 0:1], in_=pool2[0:1, 1:2])
            nc.vector.tensor_scalar(
                out=eca1, in0=shd[:], scalar1=wecab[:, 0:1], scalar2=None,
                op0=ALU.mult,
            )
            nc.vector.scalar_tensor_tensor(
                out=eca2, in0=pool2[:], scalar=wecab[:, 1:2], in1=eca1[:],
                op0=ALU.mult, op1=ALU.add,
            )
            nc.vector.scalar_tensor_tensor(
                out=eca1, in0=shu[:], scalar=wecab[:, 2:3], in1=eca2[:],
                op0=ALU.mult, op1=ALU.add,
            )

        # ---- main loop ---------------------------------------------------
        fused_t = [None] * NCHUNKS
        ypt_t = [None] * NCHUNKS
        pr_t = {}

        def emit_mm1_tap(ci, k):
            """mm1 for tap k (both ct out-tiles) + its evict/product."""
            dve_stt = k in DVE_STT_TAPS
            pr = prod_pool.tile([128, CT, CHUNK], BF16, tag=f"pr{k}",
                                name=f"pr{k}")
            fsb = None
            if not dve_stt:
                fsb = fsb_pool.tile([128, CT, CHUNK], BF16, tag="fsb",
                                    name="fsb")
            for ct in range(CT):
                mt = k * CT + ct
                fp = fps.tile([128, CHUNK], F32, tag="fp", name="fp")
                for kt in range(CT):
                    lhsT = wf_sb[kt][:, mt * 128 : (mt + 1) * 128]
                    for nh in range(NH):
                        nc.tensor.matmul(
                            fp[:, nh * 512 : (nh + 1) * 512],
                            lhsT,
                            center(ci, kt, nh),
                            start=(kt == 0),
                            stop=(kt == CT - 1),
                        )
                if dve_stt:
                    nc.vector.scalar_tensor_tensor(
                        out=pr[:, ct, :].rearrange("p (r w) -> p r w", w=W),
                        in0=fp[:].rearrange("p (r w) -> p r w", w=W),
                        scalar=bfp_sb[:, mt : mt + 1],
                        in1=win_ct(ci, k, ct),
                        op0=ALU.add, op1=ALU.mult,
                    )
                else:
                    nc.scalar.activation(
                        out=fsb[:, ct, :], in_=fp[:], func=AF.Identity,
                        bias=bfp_sb[:, mt : mt + 1], scale=1.0,
                    )
            pr_t[k] = pr
            return fsb, pr

        def emit_prod(ci, k, fsb, pr):
            if k in POOL_TAPS:
                for ct in range(CT):
                    nc.gpsimd.tensor_tensor(
                        out=pr[:, ct, :].rearrange("p (r w) -> p r w", w=W),
                        in0=fsb[:, ct, :].rearrange("p (r w) -> p r w", w=W),
                        in1=win_ct(ci, k, ct),
                        op=ALU.mult,
                    )
            else:
                nc.vector.tensor_tensor(
                    out=pr[:].rearrange("p c (r w) -> p c r w", w=W),
                    in0=fsb[:].rearrange("p c (r w) -> p c r w", w=W),
                    in1=win_all(ci, k),
                    op=ALU.mult,
                )

        def addp(a, b):
            nc.vector.tensor_add(pr_t[a][:], pr_t[a][:], pr_t[b][:])

        def emit_mm2_part(ci, mt2, nh):
            yp = yps.tile([128, 512], F32, tag="yp", name="yp")
            for kt in range(CT):
                nc.tensor.matmul(
                    yp[:],
                    wp_sb[kt][:, mt2 * 128 : (mt2 + 1) * 128],
                    fused_t[ci][:, kt, nh * 512 : (nh + 1) * 512],
                    start=(kt == 0),
                    stop=False,
                )
            for kt in range(CT):
                nc.tensor.matmul(
                    yp[:],
                    weff[kt][:, mt2 * 128 : (mt2 + 1) * 128],
                    center(ci, kt, nh),
                    start=False,
                    stop=(kt == CT - 1),
                )
            if ypt_t[ci] is None:
                ypt_t[ci] = [[None] * NH for _ in range(CT)]
            ypt_t[ci][mt2][nh] = yp

        def emit_yev(ci, mt2):
            r0 = ci * ROWS
            for nh in range(NH):
                src = ypt_t[ci][mt2][nh]
                dst = y_sb[mt2][:, r0 * W + nh * 512 : r0 * W + (nh + 1) * 512]
                nc.scalar.activation(out=dst, in_=src[:], func=AF.Copy)

        def emit_bn(ci, mt2):
            # stats from the freshly evicted bf16 y slices (frees PSUM
            # sooner than reading the mm2 PSUM tiles; bn_stats caps at 512)
            r0 = ci * ROWS
            for nh in range(NH):
                lo = r0 * W + nh * 512
                nc.vector.bn_stats(
                    out=stats_sb[mt2][:, ci * NH + nh, :],
                    in_=y_sb[mt2][:, lo : lo + 512],
                )

        def emit_weff():
            for kt in range(CT):
                nc.vector.tensor_scalar(
                    out=weff[kt][:], in0=wp_sb[kt][:],
                    scalar1=attn[:, kt : kt + 1], scalar2=None, op0=ALU.mult,
                )

        def emit_chunk(ci):
            cj = ci - 1
            first = ci == 0
            ft = prod_pool.tile([128, CT, CHUNK], BF16, tag="fused",
                                name="fused", bufs=2)
            for k in range(KS * KS):
                fsb, pr = emit_mm1_tap(ci, k)
                # hooks between mm1 and the DVE product
                if not first:
                    if k == 3:
                        emit_mm2_part(cj, 0, 0)
                    elif k == 4:
                        emit_mm2_part(cj, 0, 1)
                    elif k == 5:
                        emit_yev(cj, 0)
                        emit_bn(cj, 0)
                    elif k == 6:
                        emit_mm2_part(cj, 1, 0)
                    elif k == 7:
                        emit_mm2_part(cj, 1, 1)
                else:
                    if k == 5:
                        pool_scalar(2)
                    elif k == 7:
                        pool_scalar(3)
                if fsb is not None:
                    emit_prod(ci, k, fsb, pr)
                # readiness-ordered add tree (in-place into pr tiles):
                # pr0 accumulates the DVE-made products (0,1,2,4,7),
                # pr3 the pool-made ones (3,5,6,8)
                if k == 2:
                    addp(0, 1)
                elif k == 4:
                    addp(0, 2)
                elif k == 5:
                    addp(0, 4)
                elif k == 7:
                    addp(3, 5)
                elif k == 8:
                    addp(0, 7)
                    addp(3, 6)
                    addp(3, 8)
                    nc.vector.tensor_add(ft[:], pr_t[0][:], pr_t[3][:])
            fused_t[ci] = ft
            if not first:
                emit_yev(cj, 1)
                emit_bn(cj, 1)
            else:
                emit_eca_combine()
                # sigmoid = 1/(1+exp(-x)) with Exp on scalar (same act table)
                nc.scalar.activation(out=eca2[:], in_=eca1[:], func=AF.Exp,
                                     bias=zb[:, 0:1], scale=-1.0)
                nc.vector.tensor_scalar(
                    out=attn, in0=eca2[:], scalar1=1.0, scalar2=None,
                    op0=ALU.add,
                )
                nc.vector.reciprocal(out=attn[:], in_=attn[:])
                emit_weff()

        pool_dve(0)
        pool_dve(1)
        for ci in range(NCHUNKS):
            emit_chunk(ci)

        # last chunk's mm2 + stats
        c3 = NCHUNKS - 1
        for mt2 in range(CT):
            for nh in range(NH):
                emit_mm2_part(c3, mt2, nh)
            emit_yev(c3, mt2)
            emit_bn(c3, mt2)

        # ---- global BN stats via all-reduce -----------------------------
        ps = consts.tile([128, CT, 2], F32, tag="ps", name="ps")
        for mt2 in range(CT):
            mv = consts.tile([128, 2], F32, tag=f"mv{mt2}", name=f"mv{mt2}")
            nc.vector.bn_aggr(out=mv[:], in_=stats_sb[mt2][:])
            nc.vector.tensor_scalar(
                out=ps[:, mt2, 0:1], in0=mv[:, 0:1], scalar1=1.0, scalar2=None,
                op0=ALU.mult,
            )
            nc.vector.scalar_tensor_tensor(
                out=ps[:, mt2, 1:2], in0=mv[:, 0:1], scalar=mv[:, 0:1],
                in1=mv[:, 1:2], op0=ALU.mult, op1=ALU.add,
            )
        nc.vector.tensor_scalar(
            out=ps[:], in0=ps[:], scalar1=float(HW), scalar2=None, op0=ALU.mult
        )

        ps_b = dram.tile([128, CT * 2], F32, tag="psb", name="psb")
        gs_b = dram.tile([128, CT * 2], F32, tag="gsb", name="gsb",
                         addr_space="Shared")
        nc.sync.dma_start(out=ps_b[:], in_=ps.rearrange("p m two -> p (m two)"))
        nc.gpsimd.collective_compute(
            "AllReduce", ALU.add, replica_groups=RG,
            ins=[ps_b[:].opt()], outs=[gs_b[:].opt()],
        )
        # preload the Sqrt act table while the collective runs (the scalar
        # queue has no Copy work left after this point)
        sqscr = consts.tile([128, 1], F32, tag="sqscr", name="sqscr")
        nc.scalar.activation(out=sqscr[:], in_=zb[:], func=AF.Sqrt,
                             bias=zb[:, 0:1], scale=1.0)
        gs = consts.tile([128, CT, 2], F32, tag="gs", name="gs")
        nc.sync.dma_start(out=gs.rearrange("p m two -> p (m two)"), in_=gs_b[:])

        # ---- normalize and write out ------------------------------------
        minv = 1.0 / float(B * HW)
        mg = consts.tile([128, CT], F32, tag="mg", name="mg")
        vg = consts.tile([128, CT], F32, tag="vg", name="vg")
        rr = consts.tile([128, CT], F32, tag="rr", name="rr")
        tt = consts.tile([128, CT], F32, tag="tt", name="tt")
        ac = consts.tile([128, CT], F32, tag="ac", name="ac")
        bc = consts.tile([128, CT], F32, tag="bc", name="bc")
        nc.vector.tensor_scalar(
            out=mg[:], in0=gs[:, :, 0], scalar1=minv, scalar2=None, op0=ALU.mult
        )
        nc.vector.tensor_scalar(
            out=vg[:], in0=gs[:, :, 1], scalar1=minv, scalar2=None, op0=ALU.mult
        )
        nc.vector.tensor_tensor(out=tt[:], in0=mg[:], in1=mg[:], op=ALU.mult)
        nc.vector.tensor_tensor(out=vg[:], in0=vg[:], in1=tt[:], op=ALU.subtract)
        nc.vector.tensor_scalar(
            out=vg[:], in0=vg[:], scalar1=1.0, scalar2=BN_EPS,
            op0=ALU.mult, op1=ALU.add,
        )
        nc.scalar.activation(out=tt[:], in_=vg[:], func=AF.Sqrt,
                             bias=zb[:, 0:1], scale=1.0)
        nc.vector.reciprocal(out=rr[:], in_=tt[:])
        nc.vector.tensor_tensor(out=ac[:], in0=rr[:], in1=gam_sb[:], op=ALU.mult)
        nc.vector.tensor_tensor(out=bc[:], in0=mg[:], in1=ac[:], op=ALU.mult)
        nc.vector.tensor_tensor(out=bc[:], in0=bet_sb[:], in1=bc[:], op=ALU.subtract)

        NSL = 4
        SL = HW // NSL
        idx = 0
        for si in range(NSL):
            for mt2 in range(CT):
                sl = slice(si * SL, (si + 1) * SL)
                nc.vector.tensor_scalar(
                    out=y_sb[mt2][:, sl], in0=y_sb[mt2][:, sl],
                    scalar1=ac[:, mt2 : mt2 + 1], scalar2=bc[:, mt2 : mt2 + 1],
                    op0=ALU.mult, op1=ALU.add,
                )
                eng = nc.sync if idx % 2 == 0 else nc.scalar
                eng.dma_start(out=yout[mt2, :, sl], in_=y_sb[mt2][:, sl])
                idx += 1


_NC = None


def _build_nc(debug=False):
    nc = bacc.Bacc(
        "TRN2", target_bir_lowering=False, debug=debug, num_devices=NCORES
    )
    with tile.TileContext(nc, num_cores=NCORES) as tc:
        _emit(tc)
    nc.compile()
    return nc


def _get_nc():
    global _NC
    if _NC is None:
        _NC = _build_nc()
    return _NC


def _prep_in_maps(x, W_filter, b_filter, w_eca, W_proj, gamma, beta):
    bf = ml_dtypes.bfloat16
    x = np.asarray(x, np.float32)
    W_filter = np.asarray(W_filter, np.float32)
    b_filter = np.asarray(b_filter, np.float32)
    w_eca = np.asarray(w_eca, np.float32)
    W_proj = np.asarray(W_proj, np.float32)
    gamma = np.asarray(gamma, np.float32)
    beta = np.asarray(beta, np.float32)

    # gutter layout: row r at G + r*SW, col SW-1 of each row stays zero
    buf = np.zeros((B, C, XB), np.float32)
    xr = x.reshape(B, C, H, W)
    for r in range(H):
        buf[:, :, G + r * SW : G + r * SW + W] = xr[:, :, r, :]
    xg_h = np.ascontiguousarray(buf.reshape(B, CT, 128, XB)).astype(bf)

    # permute mm1 weights: o' = k*256 + c  (original o = c*9 + k)
    wperm = W_filter.reshape(C, KS * KS, C).transpose(1, 0, 2).reshape(KS * KS * C, C)
    wf_h = np.ascontiguousarray(wperm.T.reshape(CT, 128, MT1 * 128)).astype(bf)
    bperm = b_filter.reshape(C, KS * KS).T.reshape(KS * KS * C)

    wp_h = np.ascontiguousarray(
        (0.5 * W_proj).T.reshape(CT, 128, C).transpose(1, 0, 2).reshape(128, CT * C)
    ).astype(bf)
    misc_h = np.zeros((128, MT1 + 7), np.float32)
    misc_h[:, 0:MT1] = bperm.reshape(MT1, 128).T
    misc_h[:, MT1 : MT1 + 3] = (w_eca / float(HW)).reshape(1, 3)
    misc_h[:, MT1 + 3 : MT1 + 5] = gamma.reshape(CT, 128).T
    misc_h[:, MT1 + 5 : MT1 + 7] = beta.reshape(CT, 128).T

    in_maps = []
    for i in range(B):
        m = {
            "xg": xg_h[i],
            "wf": wf_h,
            "misc": misc_h,
            "wp": wp_h,
        }
        in_maps.append(m)
    return in_maps


last_result = None


def kernel(x, W_filter, b_filter, w_eca, W_proj, b_proj, gamma, beta):
    """Full-input, full-output DDF module on 8 NeuronCores."""
    global last_result
    # b_proj is mathematically cancelled by the batch-norm; unused.
    in_maps = _prep_in_maps(x, W_filter, b_filter, w_eca, W_proj, gamma, beta)
    nc = _get_nc()
    trace = bool(int(os.environ.get("DDF_TRACE", "0")))
    res = run_bass_kernel_spmd(nc, in_maps, list(range(NCORES)), trace=trace)
    last_result = res
    out = np.stack(
        [
            np.asarray(res.results[i]["y"]).reshape(C, H, W).astype(np.float32)
            for i in range(B)
        ]
    )
    return out


# revision 26
# speedup vs baseline: 1.0361x; 1.0361x over previous
"""Trainium2 Bass kernel for the DDF (dynamic-filter + ECA + BN) module.

Data-parallel over batch B=8 across 8 NeuronCores (one image per core),
params replicated, sync-BN via a single small AllReduce.

Layout: channels on partitions (CT=2 channel-tiles of 128); x lives in ONE
SBUF buffer with a 65-elem row stride whose 65th column is a zero "gutter".
All nine 3x3 window shifts are then plain slices of that buffer (the gutter
supplies the zero-pad at the row edges), so no derived shifted copies, no
wrap fix-ups.

Per 16-row chunk, the 18 mm1 PSUM tiles are drained by three engines in
parallel so the PE never waits on a PSUM bank:
  - Scalar taps: ACT evict (+bias) to SBUF, DVE does the bf16 window product.
  - Pool taps:   gpsimd scalar_tensor_tensor does (psum+bias)*window in one op.
The 9 tap products are summed by an 8-add DVE tree ordered by readiness, and
mm2 contracts {fused, attn*x} through W_proj (x-branch via attention-scaled
weights).  mm2 of chunk i is interleaved into mm1 of chunk i+1.  BN stats are
taken from the mm2 PSUM tiles (DVE bn_stats), aggregated, and exchanged with
one 2KB AllReduce on the sync queue; the Sqrt act-table load is hidden under
the AllReduce.  y is kept in bf16 and normalized in place, then written out
on two DMA queues.
"""

import os

import numpy as np
import ml_dtypes

import concourse.bass as bass
import concourse.mybir as mybir
import concourse.tile as tile
from concourse import bacc
from concourse.bass_utils import run_bass_kernel_spmd

B, C, H, W = 8, 256, 64, 64
KS = 3
HW = H * W                    # 4096
SW = W + 1                    # row stride with zero gutter column
G = SW + 1                    # guard elems at each end (covers di,dj = -1,-1)
XB = G + H * SW + G           # 4292 per channel-tile
NCORES = 8
CT = 2                        # channel tiles of 128
MT1 = KS * KS * CT            # 18 mm1 output m-tiles
BN_EPS = 1e-5
F32 = mybir.dt.float32
BF16 = mybir.dt.bfloat16
ROWS = 16                     # rows per chunk
NCHUNKS = H // ROWS           # 4
CHUNK = ROWS * W              # 1024 pixels per chunk per channel-tile
NH = CHUNK // 512             # 512-px matmul groups per chunk

AF = mybir.ActivationFunctionType
ALU = mybir.AluOpType
RG = [list(range(NCORES))]

# Pool cannot read PSUM on TRN2, and bulk gpsimd tensor_tensor traffic was
# measured to slow concurrent DVE ops ~2x (SBUF contention), so the drain
# pipeline uses Scalar (all 18 ACT evicts+bias) + DVE (all products, adds,
# bn) only — measured balanced at ~23us/chunk each.
POOL_TAPS = ()
DVE_STT_TAPS = ()


def _emit(tc):
    nc = tc.nc

    xgp = nc.declare_dram_parameter("xg", [CT, 128, XB], BF16, isOutput=False)
    wf = nc.declare_dram_parameter("wf", [CT, 128, MT1 * 128], BF16, isOutput=False)
    # misc fp32 params packed: bfp[18] | weca[3] | gam[2] | bet[2]
    misc = nc.declare_dram_parameter("misc", [128, MT1 + 7], F32, isOutput=False)
    wp = nc.declare_dram_parameter("wp", [128, CT * C], BF16, isOutput=False)
    yout = nc.declare_dram_parameter("y", [CT, 128, HW], BF16, isOutput=True)

    with (
        tc.tile_pool(name="consts", bufs=1) as consts,
        tc.tile_pool(name="fps", bufs=3, space="PSUM") as fps,
        tc.tile_pool(name="yps", bufs=2, space="PSUM") as yps,
        tc.tile_pool(name="fsb", bufs=5) as fsb_pool,
        tc.tile_pool(name="prod", bufs=1) as prod_pool,
        tc.tile_pool(name="dram", bufs=1, space="DRAM") as dram,
    ):
        # ---- resident tensors -------------------------------------------
        wf_sb = [consts.tile([128, MT1 * 128], BF16, tag=f"wf{kt}", name=f"wf{kt}")
                 for kt in range(CT)]
        wpb = consts.tile([128, CT, C], BF16, tag="wpb", name="wpb")
        wp_sb = [wpb[:, kt, :] for kt in range(CT)]
        weffb = consts.tile([128, CT, C], BF16, tag="weffb", name="weffb")
        weff = [weffb[:, kt, :] for kt in range(CT)]
        miscb = consts.tile([128, MT1 + 7], F32, tag="miscb", name="miscb")
        bfp_sb = miscb[:, 0:MT1]
        wecab = miscb[:, MT1 : MT1 + 3]
        gam_sb = miscb[:, MT1 + 3 : MT1 + 5]
        bet_sb = miscb[:, MT1 + 5 : MT1 + 7]
        xg = consts.tile([128, CT, XB], BF16, tag="xg", name="xg")
        y_sb = [consts.tile([128, HW], BF16, tag=f"ysb{mt}", name=f"ysb{mt}")
                for mt in range(CT)]
        stats_sb = [
            consts.tile([128, NCHUNKS * NH, 6], F32, tag=f"st{mt}", name=f"st{mt}")
            for mt in range(CT)
        ]
        pscr = consts.tile([128, ROWS * SW], F32, tag="pscr", name="pscr")
        pacc = consts.tile([128, CT, NCHUNKS], F32, tag="pacc", name="pacc")
        zb = consts.tile([128, 1], F32, tag="zb", name="zb")
        nc.vector.memset(zb[:], 0.0)

        # ---- collective warmup ------------------------------------------
        warm_in = dram.tile([128, 1], F32, tag="wi", name="wi")
        warm_out = dram.tile([128, 1], F32, tag="wo", name="wo",
                             addr_space="Shared")
        nc.sync.dma_start(out=warm_in[:], in_=zb[:])
        nc.gpsimd.collective_compute(
            "AllReduce", ALU.add, replica_groups=RG,
            ins=[warm_in[:].opt()], outs=[warm_out[:].opt()],
        )

        # ---- input DMAs --------------------------------------------------
        # x in 4 overlapping row pieces per ct (halo rows included so chunk
        # i's windows only read pieces <= i); ct0 on sync, ct1 on gpsimd.
        for i in range(NCHUNKS):
            lo = 0 if i == 0 else G + (ROWS * i - 1) * SW - 2
            hi = XB if i == NCHUNKS - 1 else G + (ROWS * i + ROWS + 1) * SW
            nc.sync.dma_start(out=xg[:, 0, lo:hi], in_=xgp[0, :, lo:hi])
            nc.gpsimd.dma_start(out=xg[:, 1, lo:hi], in_=xgp[1, :, lo:hi])
        for kt in range(CT):
            nc.scalar.dma_start(out=wf_sb[kt][:], in_=wf[kt])
        nc.scalar.dma_start(out=miscb[:], in_=misc[:, :])
        nc.scalar.dma_start(
            out=wpb.rearrange("p c x -> p (c x)"), in_=wp[:, :]
        )

        # ---- window / center access patterns ----------------------------
        def win_all(ci, k):
            di, dj = divmod(k, KS)
            off = G + (ROWS * ci + di - 1) * SW + (dj - 1)
            return xg[:, :, off : off + ROWS * SW].rearrange(
                "p c (r w) -> p c r w", w=SW)[:, :, :, 0:W]

        def win_ct(ci, k, ct):
            di, dj = divmod(k, KS)
            off = G + (ROWS * ci + di - 1) * SW + (dj - 1)
            return xg[:, ct, off : off + ROWS * SW].rearrange(
                "p (r w) -> p r w", w=SW)[:, :, 0:W]

        def center(ci, kt, nh):
            off = G + (ROWS * ci + 8 * nh) * SW
            return xg[:, kt, off : off + 8 * SW].rearrange(
                "p (r w) -> p r w", w=SW)[:, :, 0:W]

        # ---- ECA pooling -------------------------------------------------
        # pieces 0,1 on DVE (head slack), 2,3 on scalar accum (hooked)
        def pool_dve(ci):
            lo = G + ROWS * ci * SW
            nc.vector.tensor_reduce(
                out=pacc[:, :, ci : ci + 1],
                in_=xg[:, :, lo : lo + ROWS * SW],
                axis=mybir.AxisListType.X,
                op=ALU.add,
            )

        def pool_scalar(ci):
            lo = G + ROWS * ci * SW
            for ct in range(CT):
                nc.scalar.activation(
                    out=pscr[:], in_=xg[:, ct, lo : lo + ROWS * SW],
                    func=AF.Copy, accum_out=pacc[:, ct, ci : ci + 1],
                )



        pool2 = consts.tile([128, CT], F32, tag="pool2", name="pool2")
        shd = consts.tile([128, CT], F32, tag="shd", name="shd")
        shu = consts.tile([128, CT], F32, tag="shu", name="shu")
        eca1 = consts.tile([128, CT], F32, tag="eca1", name="eca1")
        eca2 = consts.tile([128, CT], F32, tag="eca2", name="eca2")
        attn = consts.tile([128, CT], F32, tag="attn", name="attn")

        def emit_eca_combine():
            nc.gpsimd.tensor_tensor(
                out=pool2[:], in0=pacc[:, :, 0], in1=pacc[:, :, 1], op=ALU.add
            )
            nc.gpsimd.tensor_tensor(
                out=pool2[:], in0=pool2[:], in1=pacc[:, :, 2], op=ALU.add
            )
            nc.gpsimd.tensor_tensor(
                out=pool2[:], in0=pool2[:], in1=pacc[:, :, 3], op=ALU.add
            )
            nc.gpsimd.memset(shd[:], 0.0)
            nc.gpsimd.memset(shu[:], 0.0)
            for ct in range(CT):
                nc.gpsimd.dma_start(
                    out=shd[1:128, ct : ct + 1], in_=pool2[0:127, ct : ct + 1]
                )
                nc.gpsimd.dma_start(
                    out=shu[0:127, ct : ct + 1], in_=pool2[1:128, ct : ct + 1]
                )
            nc.gpsimd.dma_start(out=shd[0:1, 1:2], in_=pool2[127:128, 0:1])
            nc.gpsimd.dma_start(out=shu[127:128, 0:1], in_=pool2[0:1, 1:2])
            nc.vector.tensor_scalar(
                out=eca1, in0=shd[:], scalar1=wecab[:, 0:1], scalar2=None,
                op0=ALU.mult,
            )
            nc.vector.scalar_tensor_tensor(
                out=eca2, in0=pool2[:], scalar=wecab[:, 1:2], in1=eca1[:],
                op0=ALU.mult, op1=ALU.add,
            )
            nc.vector.scalar_tensor_tensor(
                out=eca1, in0=shu[:], scalar=wecab[:, 2:3], in1=eca2[:],
                op0=ALU.mult, op1=ALU.add,
            )

        # ---- main loop ---------------------------------------------------
        fused_t = [None] * NCHUNKS
        ypt_t = [None] * NCHUNKS
        pr_t = {}

        def emit_mm1_tap(ci, k):
            """mm1 for tap k (both ct out-tiles) + its evict/product."""
            dve_stt = k in DVE_STT_TAPS
            pr = prod_pool.tile([128, CT, CHUNK], BF16, tag=f"pr{k}",
                                name=f"pr{k}")
            fsb = None
            if not dve_stt:
                fsb = fsb_pool.tile([128, CT, CHUNK], BF16, tag="fsb",
                                    name="fsb")
            for ct in range(CT):
                mt = k * CT + ct
                fp = fps.tile([128, CHUNK], F32, tag="fp", name="fp")
                for kt in range(CT):
                    lhsT = wf_sb[kt][:, mt * 128 : (mt + 1) * 128]
                    for nh in range(NH):
                        nc.tensor.matmul(
                            fp[:, nh * 512 : (nh + 1) * 512],
                            lhsT,
                            center(ci, kt, nh),
                            start=(kt == 0),
                            stop=(kt == CT - 1),
                        )
                if dve_stt:
                    nc.vector.scalar_tensor_tensor(
                        out=pr[:, ct, :].rearrange("p (r w) -> p r w", w=W),
                        in0=fp[:].rearrange("p (r w) -> p r w", w=W),
                        scalar=bfp_sb[:, mt : mt + 1],
                        in1=win_ct(ci, k, ct),
                        op0=ALU.add, op1=ALU.mult,
                    )
                else:
                    nc.scalar.activation(
                        out=fsb[:, ct, :], in_=fp[:], func=AF.Identity,
                        bias=bfp_sb[:, mt : mt + 1], scale=1.0,
                    )
            pr_t[k] = pr
            return fsb, pr

        def emit_prod(ci, k, fsb, pr):
            if k in POOL_TAPS:
                for ct in range(CT):
                    nc.gpsimd.tensor_tensor(
                        out=pr[:, ct, :].rearrange("p (r w) -> p r w", w=W),
                        in0=fsb[:, ct, :].rearrange("p (r w) -> p r w", w=W),
                        in1=win_ct(ci, k, ct),
                        op=ALU.mult,
                    )
            else:
                nc.vector.tensor_tensor(
                    out=pr[:].rearrange("p c (r w) -> p c r w", w=W),
                    in0=fsb[:].rearrange("p c (r w) -> p c r w", w=W),
                    in1=win_all(ci, k),
                    op=ALU.mult,
                )

        def addp(a, b):
            nc.vector.tensor_add(pr_t[a][:], pr_t[a][:], pr_t[b][:])

        def emit_mm2_part(ci, mt2, nh):
            yp = yps.tile([128, 512], F32, tag="yp", name="yp")
            for kt in range(CT):
                nc.tensor.matmul(
                    yp[:],
                    wp_sb[kt][:, mt2 * 128 : (mt2 + 1) * 128],
                    fused_t[ci][:, kt, nh * 512 : (nh + 1) * 512],
                    start=(kt == 0),
                    stop=False,
                )
            for kt in range(CT):
                nc.tensor.matmul(
                    yp[:],
                    weff[kt][:, mt2 * 128 : (mt2 + 1) * 128],
                    center(ci, kt, nh),
                    start=False,
                    stop=(kt == CT - 1),
                )
            if ypt_t[ci] is None:
                ypt_t[ci] = [[None] * NH for _ in range(CT)]
            ypt_t[ci][mt2][nh] = yp

        def emit_yev(ci, mt2):
            r0 = ci * ROWS
            for nh in range(NH):
                src = ypt_t[ci][mt2][nh]
                dst = y_sb[mt2][:, r0 * W + nh * 512 : r0 * W + (nh + 1) * 512]
                nc.scalar.activation(out=dst, in_=src[:], func=AF.Copy)

        def emit_bn(ci, mt2):
            # stats from the freshly evicted bf16 y slices (frees PSUM
            # sooner than reading the mm2 PSUM tiles; bn_stats caps at 512)
            r0 = ci * ROWS
            for nh in range(NH):
                lo = r0 * W + nh * 512
                nc.vector.bn_stats(
                    out=stats_sb[mt2][:, ci * NH + nh, :],
                    in_=y_sb[mt2][:, lo : lo + 512],
                )

        def emit_weff():
            for kt in range(CT):
                nc.vector.tensor_scalar(
                    out=weff[kt][:], in0=wp_sb[kt][:],
                    scalar1=attn[:, kt : kt + 1], scalar2=None, op0=ALU.mult,
                )

        def emit_chunk(ci):
            cj = ci - 1
            first = ci == 0
            ft = prod_pool.tile([128, CT, CHUNK], BF16, tag="fused",
                                name="fused", bufs=2)
            for k in range(KS * KS):
                fsb, pr = emit_mm1_tap(ci, k)
                # hooks between mm1 and the DVE product
                if not first:
                    if k == 3:
                        emit_mm2_part(cj, 0, 0)
                    elif k == 4:
                        emit_mm2_part(cj, 0, 1)
                    elif k == 5:
                        emit_yev(cj, 0)
                        emit_bn(cj, 0)
                    elif k == 6:
                        emit_mm2_part(cj, 1, 0)
                    elif k == 7:
                        emit_mm2_part(cj, 1, 1)
                else:
                    if k == 5:
                        pool_scalar(2)
                    elif k == 7:
                        pool_dve(3)
                if fsb is not None:
                    emit_prod(ci, k, fsb, pr)
                # add tree woven between taps (in-place into pr tiles)
                if k == 1:
                    addp(0, 1)
                elif k == 3:
                    addp(2, 3)
                    addp(0, 2)
                elif k == 5:
                    addp(4, 5)
                elif k == 7:
                    addp(6, 7)
                    addp(4, 6)
                elif k == 8:
                    addp(0, 4)
                    nc.vector.tensor_add(ft[:], pr_t[0][:], pr_t[8][:])
            fused_t[ci] = ft
            if not first:
                emit_yev(cj, 1)
                emit_bn(cj, 1)
            else:
                emit_eca_combine()
                # sigmoid = 1/(1+exp(-x)) with Exp on scalar (same act table)
                nc.scalar.activation(out=eca2[:], in_=eca1[:], func=AF.Exp,
                                     bias=zb[:, 0:1], scale=-1.0)
                nc.vector.tensor_scalar(
                    out=attn, in0=eca2[:], scalar1=1.0, scalar2=None,
                    op0=ALU.add,
                )
                nc.vector.reciprocal(out=attn[:], in_=attn[:])
                emit_weff()

        pool_dve(0)
        pool_dve(1)
        for ci in range(NCHUNKS):
            emit_chunk(ci)

        # last chunk's mm2 + stats
        c3 = NCHUNKS - 1
        for mt2 in range(CT):
            for nh in range(NH):
                emit_mm2_part(c3, mt2, nh)
            emit_yev(c3, mt2)
            emit_bn(c3, mt2)

        # ---- global BN stats via all-reduce -----------------------------
        ps = consts.tile([128, CT, 2], F32, tag="ps", name="ps")
        for mt2 in range(CT):
            mv = consts.tile([128, 2], F32, tag=f"mv{mt2}", name=f"mv{mt2}")
            nc.vector.bn_aggr(out=mv[:], in_=stats_sb[mt2][:])
            nc.vector.tensor_scalar(
                out=ps[:, mt2, 0:1], in0=mv[:, 0:1], scalar1=1.0, scalar2=None,
                op0=ALU.mult,
            )
            nc.vector.scalar_tensor_tensor(
                out=ps[:, mt2, 1:2], in0=mv[:, 0:1], scalar=mv[:, 0:1],
                in1=mv[:, 1:2], op0=ALU.mult, op1=ALU.add,
            )
        nc.vector.tensor_scalar(
            out=ps[:], in0=ps[:], scalar1=float(HW), scalar2=None, op0=ALU.mult
        )

        ps_b = dram.tile([128, CT * 2], F32, tag="psb", name="psb")
        gs_b = dram.tile([128, CT * 2], F32, tag="gsb", name="gsb",
                         addr_space="Shared")
        nc.sync.dma_start(out=ps_b[:], in_=ps.rearrange("p m two -> p (m two)"))
        nc.gpsimd.collective_compute(
            "AllReduce", ALU.add, replica_groups=RG,
            ins=[ps_b[:].opt()], outs=[gs_b[:].opt()],
        )
        # preload the Sqrt act table while the collective runs (the scalar
        # queue has no Copy work left after this point)
        sqscr = consts.tile([128, 1], F32, tag="sqscr", name="sqscr")
        nc.scalar.activation(out=sqscr[:], in_=zb[:], func=AF.Sqrt,
                             bias=zb[:, 0:1], scale=1.0)
        gs = consts.tile([128, CT, 2], F32, tag="gs", name="gs")
        nc.sync.dma_start(out=gs.rearrange("p m two -> p (m two)"), in_=gs_b[:])

        # ---- normalize and write out ------------------------------------
        minv = 1.0 / float(B * HW)
        mg = consts.tile([128, CT], F32, tag="mg", name="mg")
        vg = consts.tile([128, CT], F32, tag="vg", name="vg")
        rr = consts.tile([128, CT], F32, tag="rr", name="rr")
        tt = consts.tile([128, CT], F32, tag="tt", name="tt")
        ac = consts.tile([128, CT], F32, tag="ac", name="ac")
        bc = consts.tile([128, CT], F32, tag="bc", name="bc")
        nc.vector.tensor_scalar(
            out=mg[:], in0=gs[:, :, 0], scalar1=minv, scalar2=None, op0=ALU.mult
        )
        nc.vector.tensor_scalar(
            out=vg[:], in0=gs[:, :, 1], scalar1=minv, scalar2=None, op0=ALU.mult
        )
        nc.vector.tensor_tensor(out=tt[:], in0=mg[:], in1=mg[:], op=ALU.mult)
        nc.vector.tensor_tensor(out=vg[:], in0=vg[:], in1=tt[:], op=ALU.subtract)
        nc.vector.tensor_scalar(
            out=vg[:], in0=vg[:], scalar1=1.0, scalar2=BN_EPS,
            op0=ALU.mult, op1=ALU.add,
        )
        nc.scalar.activation(out=tt[:], in_=vg[:], func=AF.Sqrt,
                             bias=zb[:, 0:1], scale=1.0)
        nc.vector.reciprocal(out=rr[:], in_=tt[:])
        nc.vector.tensor_tensor(out=ac[:], in0=rr[:], in1=gam_sb[:], op=ALU.mult)
        nc.vector.tensor_tensor(out=bc[:], in0=mg[:], in1=ac[:], op=ALU.mult)
        nc.vector.tensor_tensor(out=bc[:], in0=bet_sb[:], in1=bc[:], op=ALU.subtract)

        NSL = 4
        SL = HW // NSL
        idx = 0
        for si in range(NSL):
            for mt2 in range(CT):
                sl = slice(si * SL, (si + 1) * SL)
                nc.vector.tensor_scalar(
                    out=y_sb[mt2][:, sl], in0=y_sb[mt2][:, sl],
                    scalar1=ac[:, mt2 : mt2 + 1], scalar2=bc[:, mt2 : mt2 + 1],
                    op0=ALU.mult, op1=ALU.add,
                )
                eng = nc.sync if idx % 2 == 0 else nc.scalar
                eng.dma_start(out=yout[mt2, :, sl], in_=y_sb[mt2][:, sl])
                idx += 1


_NC = None


def _build_nc(debug=False):
    nc = bacc.Bacc(
        "TRN2", target_bir_lowering=False, debug=debug, num_devices=NCORES
    )
    with tile.TileContext(nc, num_cores=NCORES) as tc:
        _emit(tc)
    nc.compile()
    return nc


def _get_nc():
    global _NC
    if _NC is None:
        _NC = _build_nc()
    return _NC


def _prep_in_maps(x, W_filter, b_filter, w_eca, W_proj, gamma, beta):
    bf = ml_dtypes.bfloat16
    x = np.asarray(x, np.float32)
    W_filter = np.asarray(W_filter, np.float32)
    b_filter = np.asarray(b_filter, np.float32)
    w_eca = np.asarray(w_eca, np.float32)
    W_proj = np.asarray(W_proj, np.float32)
    gamma = np.asarray(gamma, np.float32)
    beta = np.asarray(beta, np.float32)

    # gutter layout: row r at G + r*SW, col SW-1 of each row stays zero
    buf = np.zeros((B, C, XB), np.float32)
    xr = x.reshape(B, C, H, W)
    for r in range(H):
        buf[:, :, G + r * SW : G + r * SW + W] = xr[:, :, r, :]
    xg_h = np.ascontiguousarray(buf.reshape(B, CT, 128, XB)).astype(bf)

    # permute mm1 weights: o' = k*256 + c  (original o = c*9 + k)
    wperm = W_filter.reshape(C, KS * KS, C).transpose(1, 0, 2).reshape(KS * KS * C, C)
    wf_h = np.ascontiguousarray(wperm.T.reshape(CT, 128, MT1 * 128)).astype(bf)
    bperm = b_filter.reshape(C, KS * KS).T.reshape(KS * KS * C)

    wp_h = np.ascontiguousarray(
        (0.5 * W_proj).T.reshape(CT, 128, C).transpose(1, 0, 2).reshape(128, CT * C)
    ).astype(bf)
    misc_h = np.zeros((128, MT1 + 7), np.float32)
    misc_h[:, 0:MT1] = bperm.reshape(MT1, 128).T
    misc_h[:, MT1 : MT1 + 3] = (w_eca / float(HW)).reshape(1, 3)
    misc_h[:, MT1 + 3 : MT1 + 5] = gamma.reshape(CT, 128).T
    misc_h[:, MT1 + 5 : MT1 + 7] = beta.reshape(CT, 128).T

    in_maps = []
    for i in range(B):
        m = {
            "xg": xg_h[i],
            "wf": wf_h,
            "misc": misc_h,
            "wp": wp_h,
        }
        in_maps.append(m)
    return in_maps


last_result = None


def kernel(x, W_filter, b_filter, w_eca, W_proj, b_proj, gamma, beta):
    """Full-input, full-output DDF module on 8 NeuronCores."""
    global last_result
    # b_proj is mathematically cancelled by the batch-norm; unused.
    in_maps = _prep_in_maps(x, W_filter, b_filter, w_eca, W_proj, gamma, beta)
    nc = _get_nc()
    trace = bool(int(os.environ.get("DDF_TRACE", "0")))
    res = run_bass_kernel_spmd(nc, in_maps, list(range(NCORES)), trace=trace)
    last_result = res
    out = np.stack(
        [
            np.asarray(res.results[i]["y"]).reshape(C, H, W).astype(np.float32)
            for i in range(B)
        ]
    )
    return out


# revision 31
# speedup vs baseline: 1.0400x; 1.0038x over previous
"""Trainium2 Bass kernel for the DDF (dynamic-filter + ECA + BN) module.

Data-parallel over batch B=8 across 8 NeuronCores (one image per core),
params replicated, sync-BN via a single small AllReduce.

Layout: channels on partitions (CT=2 channel-tiles of 128); x lives in ONE
SBUF buffer with a 65-elem row stride whose 65th column is a zero "gutter".
All nine 3x3 window shifts are then plain slices of that buffer (the gutter
supplies the zero-pad at the row edges), so no derived shifted copies, no
wrap fix-ups.

Per 16-row chunk, the 18 mm1 PSUM tiles are drained by three engines in
parallel so the PE never waits on a PSUM bank:
  - Scalar taps: ACT evict (+bias) to SBUF, DVE does the bf16 window product.
  - Pool taps:   gpsimd scalar_tensor_tensor does (psum+bias)*window in one op.
The 9 tap products are summed by an 8-add DVE tree ordered by readiness, and
mm2 contracts {fused, attn*x} through W_proj (x-branch via attention-scaled
weights).  mm2 of chunk i is interleaved into mm1 of chunk i+1.  BN stats are
taken from the mm2 PSUM tiles (DVE bn_stats), aggregated, and exchanged with
one 2KB AllReduce on the sync queue; the Sqrt act-table load is hidden under
the AllReduce.  y is kept in bf16 and normalized in place, then written out
on two DMA queues.
"""

import os

import numpy as np
import ml_dtypes

import concourse.bass as bass
import concourse.mybir as mybir
import concourse.tile as tile
from concourse import bacc
from concourse.bass_utils import run_bass_kernel_spmd

B, C, H, W = 8, 256, 64, 64
KS = 3
HW = H * W                    # 4096
SW = W + 1                    # row stride with zero gutter column
G = SW + 1                    # guard elems at each end (covers di,dj = -1,-1)
XB = G + H * SW + G           # 4292 per channel-tile
NCORES = 8
CT = 2                        # channel tiles of 128
MT1 = KS * KS * CT            # 18 mm1 output m-tiles
BN_EPS = 1e-5
F32 = mybir.dt.float32
BF16 = mybir.dt.bfloat16
ROWS = 16                     # rows per chunk
NCHUNKS = H // ROWS           # 4
CHUNK = ROWS * W              # 1024 pixels per chunk per channel-tile
NH = CHUNK // 512             # 512-px matmul groups per chunk

AF = mybir.ActivationFunctionType
ALU = mybir.AluOpType
RG = [list(range(NCORES))]

# Pool cannot read PSUM on TRN2, and bulk gpsimd tensor_tensor traffic was
# measured to slow concurrent DVE ops ~2x (SBUF contention), so the drain
# pipeline uses Scalar (all 18 ACT evicts+bias) + DVE (all products, adds,
# bn) only — measured balanced at ~23us/chunk each.
POOL_TAPS = ()
DVE_STT_TAPS = (8,)           # drain tap 8 on DVE to shorten the chunk-end
                              # evict->product->fused critical chain


def _emit(tc):
    nc = tc.nc

    xgp = nc.declare_dram_parameter("xg", [CT, 128, XB], BF16, isOutput=False)
    wf = nc.declare_dram_parameter("wf", [CT, 128, MT1 * 128], BF16, isOutput=False)
    # misc fp32 params packed: bfp[18] | weca[3] | gam[2] | bet[2]
    misc = nc.declare_dram_parameter("misc", [128, MT1 + 7], F32, isOutput=False)
    wp = nc.declare_dram_parameter("wp", [128, CT * C], BF16, isOutput=False)
    yout = nc.declare_dram_parameter("y", [CT, 128, HW], BF16, isOutput=True)

    with (
        tc.tile_pool(name="consts", bufs=1) as consts,
        tc.tile_pool(name="fps", bufs=3, space="PSUM") as fps,
        tc.tile_pool(name="yps", bufs=2, space="PSUM") as yps,
        tc.tile_pool(name="fsb", bufs=5) as fsb_pool,
        tc.tile_pool(name="prod", bufs=1) as prod_pool,
        tc.tile_pool(name="dram", bufs=1, space="DRAM") as dram,
    ):
        # ---- resident tensors -------------------------------------------
        wf_sb = [consts.tile([128, MT1 * 128], BF16, tag=f"wf{kt}", name=f"wf{kt}")
                 for kt in range(CT)]
        wpb = consts.tile([128, CT, C], BF16, tag="wpb", name="wpb")
        wp_sb = [wpb[:, kt, :] for kt in range(CT)]
        weffb = consts.tile([128, CT, C], BF16, tag="weffb", name="weffb")
        weff = [weffb[:, kt, :] for kt in range(CT)]
        miscb = consts.tile([128, MT1 + 7], F32, tag="miscb", name="miscb")
        bfp_sb = miscb[:, 0:MT1]
        wecab = miscb[:, MT1 : MT1 + 3]
        gam_sb = miscb[:, MT1 + 3 : MT1 + 5]
        bet_sb = miscb[:, MT1 + 5 : MT1 + 7]
        xg = consts.tile([128, CT, XB], BF16, tag="xg", name="xg")
        y_sb = [consts.tile([128, HW], BF16, tag=f"ysb{mt}", name=f"ysb{mt}")
                for mt in range(CT)]
        stats_sb = [
            consts.tile([128, NCHUNKS * NH, 6], F32, tag=f"st{mt}", name=f"st{mt}")
            for mt in range(CT)
        ]
        pscr = consts.tile([128, ROWS * SW], F32, tag="pscr", name="pscr")
        pacc = consts.tile([128, CT, NCHUNKS], F32, tag="pacc", name="pacc")
        zb = consts.tile([128, 1], F32, tag="zb", name="zb")
        nc.vector.memset(zb[:], 0.0)

        # ---- collective warmup ------------------------------------------
        warm_in = dram.tile([128, 1], F32, tag="wi", name="wi")
        warm_out = dram.tile([128, 1], F32, tag="wo", name="wo",
                             addr_space="Shared")
        nc.sync.dma_start(out=warm_in[:], in_=zb[:])
        nc.gpsimd.collective_compute(
            "AllReduce", ALU.add, replica_groups=RG,
            ins=[warm_in[:].opt()], outs=[warm_out[:].opt()],
        )

        # ---- input DMAs --------------------------------------------------
        # x in 4 overlapping row pieces per ct (halo rows included so chunk
        # i's windows only read pieces <= i); ct0 on sync, ct1 on gpsimd.
        # wf tiles split scalar/sync so both land within ~2.5us.
        pieces = []
        for i in range(NCHUNKS):
            lo = 0 if i == 0 else G + (ROWS * i - 1) * SW - 2
            hi = XB if i == NCHUNKS - 1 else G + (ROWS * i + ROWS + 1) * SW
            pieces.append((lo, hi))
        nc.sync.dma_start(out=xg[:, 0, pieces[0][0]:pieces[0][1]],
                          in_=xgp[0, :, pieces[0][0]:pieces[0][1]])
        nc.gpsimd.dma_start(out=xg[:, 1, pieces[0][0]:pieces[0][1]],
                            in_=xgp[1, :, pieces[0][0]:pieces[0][1]])
        nc.scalar.dma_start(out=wf_sb[0][:], in_=wf[0])
        nc.sync.dma_start(out=wf_sb[1][:], in_=wf[1])
        for lo, hi in pieces[1:]:
            nc.sync.dma_start(out=xg[:, 0, lo:hi], in_=xgp[0, :, lo:hi])
            nc.gpsimd.dma_start(out=xg[:, 1, lo:hi], in_=xgp[1, :, lo:hi])
        nc.scalar.dma_start(out=miscb[:], in_=misc[:, :])
        nc.scalar.dma_start(
            out=wpb.rearrange("p c x -> p (c x)"), in_=wp[:, :]
        )

        # ---- window / center access patterns ----------------------------
        def win_all(ci, k):
            di, dj = divmod(k, KS)
            off = G + (ROWS * ci + di - 1) * SW + (dj - 1)
            return xg[:, :, off : off + ROWS * SW].rearrange(
                "p c (r w) -> p c r w", w=SW)[:, :, :, 0:W]

        def win_ct(ci, k, ct):
            di, dj = divmod(k, KS)
            off = G + (ROWS * ci + di - 1) * SW + (dj - 1)
            return xg[:, ct, off : off + ROWS * SW].rearrange(
                "p (r w) -> p r w", w=SW)[:, :, 0:W]

        def center(ci, kt, nh):
            off = G + (ROWS * ci + 8 * nh) * SW
            return xg[:, kt, off : off + 8 * SW].rearrange(
                "p (r w) -> p r w", w=SW)[:, :, 0:W]

        # ---- ECA pooling -------------------------------------------------
        # pieces 0,1 on DVE (head slack), 2,3 on scalar accum (hooked)
        def pool_dve(ci):
            lo = G + ROWS * ci * SW
            nc.vector.tensor_reduce(
                out=pacc[:, :, ci : ci + 1],
                in_=xg[:, :, lo : lo + ROWS * SW],
                axis=mybir.AxisListType.X,
                op=ALU.add,
            )

        def pool_scalar(ci):
            lo = G + ROWS * ci * SW
            for ct in range(CT):
                nc.scalar.activation(
                    out=pscr[:], in_=xg[:, ct, lo : lo + ROWS * SW],
                    func=AF.Copy, accum_out=pacc[:, ct, ci : ci + 1],
                )



        pool2 = consts.tile([128, CT], F32, tag="pool2", name="pool2")
        shd = consts.tile([128, CT], F32, tag="shd", name="shd")
        shu = consts.tile([128, CT], F32, tag="shu", name="shu")
        eca1 = consts.tile([128, CT], F32, tag="eca1", name="eca1")
        eca2 = consts.tile([128, CT], F32, tag="eca2", name="eca2")
        attn = consts.tile([128, CT], F32, tag="attn", name="attn")

        def emit_eca_combine():
            nc.gpsimd.tensor_tensor(
                out=pool2[:], in0=pacc[:, :, 0], in1=pacc[:, :, 1], op=ALU.add
            )
            nc.gpsimd.tensor_tensor(
                out=pool2[:], in0=pool2[:], in1=pacc[:, :, 2], op=ALU.add
            )
            nc.gpsimd.tensor_tensor(
                out=pool2[:], in0=pool2[:], in1=pacc[:, :, 3], op=ALU.add
            )
            nc.gpsimd.memset(shd[:], 0.0)
            nc.gpsimd.memset(shu[:], 0.0)
            for ct in range(CT):
                nc.gpsimd.dma_start(
                    out=shd[1:128, ct : ct + 1], in_=pool2[0:127, ct : ct + 1]
                )
                nc.gpsimd.dma_start(
                    out=shu[0:127, ct : ct + 1], in_=pool2[1:128, ct : ct + 1]
                )
            nc.gpsimd.dma_start(out=shd[0:1, 1:2], in_=pool2[127:128, 0:1])
            nc.gpsimd.dma_start(out=shu[127:128, 0:1], in_=pool2[0:1, 1:2])
            nc.vector.tensor_scalar(
                out=eca1, in0=shd[:], scalar1=wecab[:, 0:1], scalar2=None,
                op0=ALU.mult,
            )
            nc.vector.scalar_tensor_tensor(
                out=eca2, in0=pool2[:], scalar=wecab[:, 1:2], in1=eca1[:],
                op0=ALU.mult, op1=ALU.add,
            )
            nc.vector.scalar_tensor_tensor(
                out=eca1, in0=shu[:], scalar=wecab[:, 2:3], in1=eca2[:],
                op0=ALU.mult, op1=ALU.add,
            )

        # ---- main loop ---------------------------------------------------
        fused_t = [None] * NCHUNKS
        ypt_t = [None] * NCHUNKS
        pr_t = {}

        def emit_mm1_tap(ci, k):
            """mm1 for tap k (both ct out-tiles) + its evict/product."""
            dve_stt = k in DVE_STT_TAPS
            pr = prod_pool.tile([128, CT, CHUNK], BF16, tag=f"pr{k}",
                                name=f"pr{k}")
            fsb = None
            if not dve_stt:
                fsb = fsb_pool.tile([128, CT, CHUNK], BF16, tag="fsb",
                                    name="fsb")
            for ct in range(CT):
                mt = k * CT + ct
                fp = fps.tile([128, CHUNK], F32, tag="fp", name="fp")
                for kt in range(CT):
                    lhsT = wf_sb[kt][:, mt * 128 : (mt + 1) * 128]
                    for nh in range(NH):
                        nc.tensor.matmul(
                            fp[:, nh * 512 : (nh + 1) * 512],
                            lhsT,
                            center(ci, kt, nh),
                            start=(kt == 0),
                            stop=(kt == CT - 1),
                        )
                if dve_stt:
                    nc.vector.scalar_tensor_tensor(
                        out=pr[:, ct, :].rearrange("p (r w) -> p r w", w=W),
                        in0=fp[:].rearrange("p (r w) -> p r w", w=W),
                        scalar=bfp_sb[:, mt : mt + 1],
                        in1=win_ct(ci, k, ct),
                        op0=ALU.add, op1=ALU.mult,
                    )
                else:
                    nc.scalar.activation(
                        out=fsb[:, ct, :], in_=fp[:], func=AF.Identity,
                        bias=bfp_sb[:, mt : mt + 1], scale=1.0,
                    )
            pr_t[k] = pr
            return fsb, pr

        def emit_prod(ci, k, fsb, pr):
            if k in POOL_TAPS:
                for ct in range(CT):
                    nc.gpsimd.tensor_tensor(
                        out=pr[:, ct, :].rearrange("p (r w) -> p r w", w=W),
                        in0=fsb[:, ct, :].rearrange("p (r w) -> p r w", w=W),
                        in1=win_ct(ci, k, ct),
                        op=ALU.mult,
                    )
            else:
                nc.vector.tensor_tensor(
                    out=pr[:].rearrange("p c (r w) -> p c r w", w=W),
                    in0=fsb[:].rearrange("p c (r w) -> p c r w", w=W),
                    in1=win_all(ci, k),
                    op=ALU.mult,
                )

        def addp(a, b):
            nc.vector.tensor_add(pr_t[a][:], pr_t[a][:], pr_t[b][:])

        def emit_mm2_part(ci, mt2, nh):
            yp = yps.tile([128, 512], F32, tag="yp", name="yp")
            for kt in range(CT):
                nc.tensor.matmul(
                    yp[:],
                    wp_sb[kt][:, mt2 * 128 : (mt2 + 1) * 128],
                    fused_t[ci][:, kt, nh * 512 : (nh + 1) * 512],
                    start=(kt == 0),
                    stop=False,
                )
            for kt in range(CT):
                nc.tensor.matmul(
                    yp[:],
                    weff[kt][:, mt2 * 128 : (mt2 + 1) * 128],
                    center(ci, kt, nh),
                    start=False,
                    stop=(kt == CT - 1),
                )
            if ypt_t[ci] is None:
                ypt_t[ci] = [[None] * NH for _ in range(CT)]
            ypt_t[ci][mt2][nh] = yp

        def emit_yev(ci, mt2):
            r0 = ci * ROWS
            for nh in range(NH):
                src = ypt_t[ci][mt2][nh]
                dst = y_sb[mt2][:, r0 * W + nh * 512 : r0 * W + (nh + 1) * 512]
                nc.scalar.activation(out=dst, in_=src[:], func=AF.Copy)

        def emit_bn(ci, mt2):
            # stats from the freshly evicted bf16 y slices (frees PSUM
            # sooner than reading the mm2 PSUM tiles; bn_stats caps at 512)
            r0 = ci * ROWS
            for nh in range(NH):
                lo = r0 * W + nh * 512
                nc.vector.bn_stats(
                    out=stats_sb[mt2][:, ci * NH + nh, :],
                    in_=y_sb[mt2][:, lo : lo + 512],
                )

        def emit_weff():
            for kt in range(CT):
                nc.vector.tensor_scalar(
                    out=weff[kt][:], in0=wp_sb[kt][:],
                    scalar1=attn[:, kt : kt + 1], scalar2=None, op0=ALU.mult,
                )

        def emit_chunk(ci):
            cj = ci - 1
            first = ci == 0
            ft = prod_pool.tile([128, CT, CHUNK], BF16, tag="fused",
                                name="fused", bufs=2)
            for k in range(KS * KS):
                fsb, pr = emit_mm1_tap(ci, k)
                # hooks between mm1 and the DVE product
                if not first:
                    if k == 3:
                        emit_mm2_part(cj, 0, 0)
                    elif k == 4:
                        emit_mm2_part(cj, 0, 1)
                    elif k == 5:
                        emit_yev(cj, 0)
                        emit_bn(cj, 0)
                    elif k == 6:
                        emit_mm2_part(cj, 1, 0)
                    elif k == 7:
                        emit_mm2_part(cj, 1, 1)
                else:
                    if k == 5:
                        pool_scalar(2)
                    elif k == 7:
                        pool_dve(3)
                if fsb is not None:
                    emit_prod(ci, k, fsb, pr)
                # add tree woven between taps (in-place into pr tiles)
                if k == 1:
                    addp(0, 1)
                elif k == 3:
                    addp(2, 3)
                    addp(0, 2)
                elif k == 5:
                    addp(4, 5)
                elif k == 7:
                    addp(6, 7)
                    addp(4, 6)
                elif k == 8:
                    addp(0, 4)
                    nc.vector.tensor_add(ft[:], pr_t[0][:], pr_t[8][:])
            fused_t[ci] = ft
            if not first:
                emit_yev(cj, 1)
                emit_bn(cj, 1)
            else:
                emit_eca_combine()
                # sigmoid = 1/(1+exp(-x)) with Exp on scalar (same act table)
                nc.scalar.activation(out=eca2[:], in_=eca1[:], func=AF.Exp,
                                     bias=zb[:, 0:1], scale=-1.0)
                nc.vector.tensor_scalar(
                    out=attn, in0=eca2[:], scalar1=1.0, scalar2=None,
                    op0=ALU.add,
                )
                nc.vector.reciprocal(out=attn[:], in_=attn[:])
                emit_weff()

        pool_dve(0)
        pool_dve(1)
        for ci in range(NCHUNKS):
            emit_chunk(ci)

        # last chunk's mm2 + stats; aggregate each mt2 as soon as its
        # stats are complete so only the pack+dma remain after mm2(3,1,*)
        ps = consts.tile([128, CT, 2], F32, tag="ps", name="ps")

        def emit_aggr(mt2):
            mv = consts.tile([128, 2], F32, tag=f"mv{mt2}", name=f"mv{mt2}")
            nc.vector.bn_aggr(out=mv[:], in_=stats_sb[mt2][:])
            nc.vector.tensor_scalar(
                out=ps[:, mt2, 0:1], in0=mv[:, 0:1], scalar1=float(HW),
                scalar2=None, op0=ALU.mult,
            )
            nc.vector.scalar_tensor_tensor(
                out=ps[:, mt2, 1:2], in0=mv[:, 0:1], scalar=mv[:, 0:1],
                in1=mv[:, 1:2], op0=ALU.mult, op1=ALU.add,
            )
            nc.vector.tensor_scalar(
                out=ps[:, mt2, 1:2], in0=ps[:, mt2, 1:2], scalar1=float(HW),
                scalar2=None, op0=ALU.mult,
            )

        c3 = NCHUNKS - 1
        for mt2 in range(CT):
            for nh in range(NH):
                emit_mm2_part(c3, mt2, nh)
            emit_yev(c3, mt2)
            emit_bn(c3, mt2)
            emit_aggr(mt2)

        ps_b = dram.tile([128, CT * 2], F32, tag="psb", name="psb")
        gs_b = dram.tile([128, CT * 2], F32, tag="gsb", name="gsb",
                         addr_space="Shared")
        nc.sync.dma_start(out=ps_b[:], in_=ps.rearrange("p m two -> p (m two)"))
        nc.gpsimd.collective_compute(
            "AllReduce", ALU.add, replica_groups=RG,
            ins=[ps_b[:].opt()], outs=[gs_b[:].opt()],
        )
        # preload the Sqrt act table while the collective runs; input is a
        # ps slice so the dependency-driven scheduler cannot hoist the
        # table switch ahead of the Identity/Copy evicts
        sqscr = consts.tile([128, 1], F32, tag="sqscr", name="sqscr")
        nc.scalar.activation(out=sqscr[:], in_=ps[:, 0, 0:1], func=AF.Sqrt,
                             bias=zb[:, 0:1], scale=1.0)
        gs = consts.tile([128, CT, 2], F32, tag="gs", name="gs")
        nc.sync.dma_start(out=gs.rearrange("p m two -> p (m two)"), in_=gs_b[:])

        # ---- normalize and write out ------------------------------------
        minv = 1.0 / float(B * HW)
        mg = consts.tile([128, CT], F32, tag="mg", name="mg")
        vg = consts.tile([128, CT], F32, tag="vg", name="vg")
        rr = consts.tile([128, CT], F32, tag="rr", name="rr")
        tt = consts.tile([128, CT], F32, tag="tt", name="tt")
        ac = consts.tile([128, CT], F32, tag="ac", name="ac")
        bc = consts.tile([128, CT], F32, tag="bc", name="bc")
        nc.vector.tensor_scalar(
            out=mg[:], in0=gs[:, :, 0], scalar1=minv, scalar2=None, op0=ALU.mult
        )
        nc.vector.tensor_scalar(
            out=vg[:], in0=gs[:, :, 1], scalar1=minv, scalar2=None, op0=ALU.mult
        )
        nc.vector.tensor_tensor(out=tt[:], in0=mg[:], in1=mg[:], op=ALU.mult)
        nc.vector.tensor_tensor(out=vg[:], in0=vg[:], in1=tt[:], op=ALU.subtract)
        nc.vector.tensor_scalar(
            out=vg[:], in0=vg[:], scalar1=1.0, scalar2=BN_EPS,
            op0=ALU.mult, op1=ALU.add,
        )
        nc.scalar.activation(out=tt[:], in_=vg[:], func=AF.Sqrt,
                             bias=zb[:, 0:1], scale=1.0)
        nc.vector.reciprocal(out=rr[:], in_=tt[:])
        nc.vector.tensor_tensor(out=ac[:], in0=rr[:], in1=gam_sb[:], op=ALU.mult)
        nc.vector.tensor_tensor(out=bc[:], in0=mg[:], in1=ac[:], op=ALU.mult)
        nc.vector.tensor_tensor(out=bc[:], in0=bet_sb[:], in1=bc[:], op=ALU.subtract)

        NSL = 4
        SL = HW // NSL
        idx = 0
        for si in range(NSL):
            for mt2 in range(CT):
                sl = slice(si * SL, (si + 1) * SL)
                nc.vector.tensor_scalar(
                    out=y_sb[mt2][:, sl], in0=y_sb[mt2][:, sl],
                    scalar1=ac[:, mt2 : mt2 + 1], scalar2=bc[:, mt2 : mt2 + 1],
                    op0=ALU.mult, op1=ALU.add,
                )
                eng = nc.sync if idx % 2 == 0 else nc.scalar
                eng.dma_start(out=yout[mt2, :, sl], in_=y_sb[mt2][:, sl])
                idx += 1


_NC = None


def _build_nc(debug=False):
    nc = bacc.Bacc(
        "TRN2", target_bir_lowering=False, debug=debug, num_devices=NCORES
    )
    with tile.TileContext(nc, num_cores=NCORES) as tc:
        _emit(tc)
    nc.compile()
    return nc


def _get_nc():
    global _NC
    if _NC is None:
        _NC = _build_nc()
    return _NC


def _prep_in_maps(x, W_filter, b_filter, w_eca, W_proj, gamma, beta):
    bf = ml_dtypes.bfloat16
    x = np.asarray(x, np.float32)
    W_filter = np.asarray(W_filter, np.float32)
    b_filter = np.asarray(b_filter, np.float32)
    w_eca = np.asarray(w_eca, np.float32)
    W_proj = np.asarray(W_proj, np.float32)
    gamma = np.asarray(gamma, np.float32)
    beta = np.asarray(beta, np.float32)

    # gutter layout: row r at G + r*SW, col SW-1 of each row stays zero
    buf = np.zeros((B, C, XB), np.float32)
    xr = x.reshape(B, C, H, W)
    for r in range(H):
        buf[:, :, G + r * SW : G + r * SW + W] = xr[:, :, r, :]
    xg_h = np.ascontiguousarray(buf.reshape(B, CT, 128, XB)).astype(bf)

    # permute mm1 weights: o' = k*256 + c  (original o = c*9 + k)
    wperm = W_filter.reshape(C, KS * KS, C).transpose(1, 0, 2).reshape(KS * KS * C, C)
    wf_h = np.ascontiguousarray(wperm.T.reshape(CT, 128, MT1 * 128)).astype(bf)
    bperm = b_filter.reshape(C, KS * KS).T.reshape(KS * KS * C)

    wp_h = np.ascontiguousarray(
        (0.5 * W_proj).T.reshape(CT, 128, C).transpose(1, 0, 2).reshape(128, CT * C)
    ).astype(bf)
    misc_h = np.zeros((128, MT1 + 7), np.float32)
    misc_h[:, 0:MT1] = bperm.reshape(MT1, 128).T
    misc_h[:, MT1 : MT1 + 3] = (w_eca / float(HW)).reshape(1, 3)
    misc_h[:, MT1 + 3 : MT1 + 5] = gamma.reshape(CT, 128).T
    misc_h[:, MT1 + 5 : MT1 + 7] = beta.reshape(CT, 128).T

    in_maps = []
    for i in range(B):
        m = {
            "xg": xg_h[i],
            "wf": wf_h,
            "misc": misc_h,
            "wp": wp_h,
        }
        in_maps.append(m)
    return in_maps


last_result = None


def kernel(x, W_filter, b_filter, w_eca, W_proj, b_proj, gamma, beta):
    """Full-input, full-output DDF module on 8 NeuronCores."""
    global last_result
    # b_proj is mathematically cancelled by the batch-norm; unused.
    in_maps = _prep_in_maps(x, W_filter, b_filter, w_eca, W_proj, gamma, beta)
    nc = _get_nc()
    trace = bool(int(os.environ.get("DDF_TRACE", "0")))
    res = run_bass_kernel_spmd(nc, in_maps, list(range(NCORES)), trace=trace)
    last_result = res
    out = np.stack(
        [
            np.asarray(res.results[i]["y"]).reshape(C, H, W).astype(np.float32)
            for i in range(B)
        ]
    )
    return out


# revision 36
# speedup vs baseline: 1.0421x; 1.0020x over previous
"""Trainium2 Bass kernel for the DDF (dynamic-filter + ECA + BN) module.

Data-parallel over batch B=8 across 8 NeuronCores (one image per core),
params replicated, sync-BN via a single small AllReduce.

Layout: channels on partitions (CT=2 channel-tiles of 128); x lives in ONE
SBUF buffer with a 65-elem row stride whose 65th column is a zero "gutter".
All nine 3x3 window shifts are then plain slices of that buffer (the gutter
supplies the zero-pad at the row edges), so no derived shifted copies, no
wrap fix-ups.

Per 16-row chunk, the 18 mm1 PSUM tiles are drained by three engines in
parallel so the PE never waits on a PSUM bank:
  - Scalar taps: ACT evict (+bias) to SBUF, DVE does the bf16 window product.
  - Pool taps:   gpsimd scalar_tensor_tensor does (psum+bias)*window in one op.
The 9 tap products are summed by an 8-add DVE tree ordered by readiness, and
mm2 contracts {fused, attn*x} through W_proj (x-branch via attention-scaled
weights).  mm2 of chunk i is interleaved into mm1 of chunk i+1.  BN stats are
taken from the mm2 PSUM tiles (DVE bn_stats), aggregated, and exchanged with
one 2KB AllReduce on the sync queue; the Sqrt act-table load is hidden under
the AllReduce.  y is kept in bf16 and normalized in place, then written out
on two DMA queues.
"""

import os

import numpy as np
import ml_dtypes

import concourse.bass as bass
import concourse.mybir as mybir
import concourse.tile as tile
from concourse import bacc
from concourse.bass_utils import run_bass_kernel_spmd

B, C, H, W = 8, 256, 64, 64
KS = 3
HW = H * W                    # 4096
SW = W + 1                    # row stride with zero gutter column
G = SW + 1                    # guard elems at each end (covers di,dj = -1,-1)
XB = G + H * SW + G           # 4292 per channel-tile
NCORES = 8
CT = 2                        # channel tiles of 128
MT1 = KS * KS * CT            # 18 mm1 output m-tiles
BN_EPS = 1e-5
F32 = mybir.dt.float32
BF16 = mybir.dt.bfloat16
ROWS = 16                     # rows per chunk
NCHUNKS = H // ROWS           # 4
CHUNK = ROWS * W              # 1024 pixels per chunk per channel-tile
NH = CHUNK // 512             # 512-px matmul groups per chunk

AF = mybir.ActivationFunctionType
ALU = mybir.AluOpType
RG = [list(range(NCORES))]

# Pool cannot read PSUM on TRN2, and bulk gpsimd tensor_tensor traffic was
# measured to slow concurrent DVE ops ~2x (SBUF contention), so the drain
# pipeline uses Scalar (all 18 ACT evicts+bias) + DVE (all products, adds,
# bn) only — measured balanced at ~23us/chunk each.
POOL_TAPS = ()
DVE_STT_TAPS = (8,)           # drain tap 8 on DVE to shorten the chunk-end
                              # evict->product->fused critical chain


def _emit(tc):
    nc = tc.nc

    xgp = nc.declare_dram_parameter("xg", [CT, 128, XB], BF16, isOutput=False)
    wf = nc.declare_dram_parameter("wf", [CT, 128, MT1 * 128], BF16, isOutput=False)
    # misc fp32 params packed: bfp[18] | weca[3] | gam[2] | bet[2]
    misc = nc.declare_dram_parameter("misc", [128, MT1 + 7], F32, isOutput=False)
    wp = nc.declare_dram_parameter("wp", [128, CT * C], BF16, isOutput=False)
    yout = nc.declare_dram_parameter("y", [CT, 128, HW], BF16, isOutput=True)

    with (
        tc.tile_pool(name="consts", bufs=1) as consts,
        tc.tile_pool(name="fps", bufs=3, space="PSUM") as fps,
        tc.tile_pool(name="yps", bufs=2, space="PSUM") as yps,
        tc.tile_pool(name="fsb", bufs=5) as fsb_pool,
        tc.tile_pool(name="prod", bufs=1) as prod_pool,
        tc.tile_pool(name="dram", bufs=1, space="DRAM") as dram,
    ):
        # ---- resident tensors -------------------------------------------
        wf_sb = [consts.tile([128, MT1 * 128], BF16, tag=f"wf{kt}", name=f"wf{kt}")
                 for kt in range(CT)]
        wpb = consts.tile([128, CT, C], BF16, tag="wpb", name="wpb")
        wp_sb = [wpb[:, kt, :] for kt in range(CT)]
        weffb = consts.tile([128, CT, C], BF16, tag="weffb", name="weffb")
        weff = [weffb[:, kt, :] for kt in range(CT)]
        miscb = consts.tile([128, MT1 + 7], F32, tag="miscb", name="miscb")
        bfp_sb = miscb[:, 0:MT1]
        wecab = miscb[:, MT1 : MT1 + 3]
        gam_sb = miscb[:, MT1 + 3 : MT1 + 5]
        bet_sb = miscb[:, MT1 + 5 : MT1 + 7]
        xg = consts.tile([128, CT, XB], BF16, tag="xg", name="xg")
        y_sb = [consts.tile([128, HW], BF16, tag=f"ysb{mt}", name=f"ysb{mt}")
                for mt in range(CT)]
        stats_sb = [
            consts.tile([128, NCHUNKS * NH, 6], F32, tag=f"st{mt}", name=f"st{mt}")
            for mt in range(CT)
        ]
        pscr = consts.tile([128, ROWS * SW], F32, tag="pscr", name="pscr")
        pacc = consts.tile([128, CT, NCHUNKS], F32, tag="pacc", name="pacc")
        zb = consts.tile([128, 1], F32, tag="zb", name="zb")
        nc.vector.memset(zb[:], 0.0)

        warm_in = dram.tile([128, 1], F32, tag="wi", name="wi")
        warm_out = dram.tile([128, 1], F32, tag="wo", name="wo",
                             addr_space="Shared")

        # ---- input DMAs --------------------------------------------------
        # x in 4 overlapping row pieces per ct (halo rows included so chunk
        # i's windows only read pieces <= i); ct0 on sync, ct1 on gpsimd.
        # wf tiles split scalar/sync so both land within ~2.5us.
        pieces = []
        for i in range(NCHUNKS):
            lo = 0 if i == 0 else G + (ROWS * i - 1) * SW - 2
            hi = XB if i == NCHUNKS - 1 else G + (ROWS * i + ROWS + 1) * SW
            pieces.append((lo, hi))
        nc.sync.dma_start(out=xg[:, 0, pieces[0][0]:pieces[0][1]],
                          in_=xgp[0, :, pieces[0][0]:pieces[0][1]])
        nc.gpsimd.dma_start(out=xg[:, 1, pieces[0][0]:pieces[0][1]],
                            in_=xgp[1, :, pieces[0][0]:pieces[0][1]])
        nc.scalar.dma_start(out=wf_sb[0][:], in_=wf[0])
        nc.sync.dma_start(out=wf_sb[1][:], in_=wf[1])
        for lo, hi in pieces[1:]:
            nc.sync.dma_start(out=xg[:, 0, lo:hi], in_=xgp[0, :, lo:hi])
            nc.gpsimd.dma_start(out=xg[:, 1, lo:hi], in_=xgp[1, :, lo:hi])
        nc.scalar.dma_start(out=miscb[:], in_=misc[:, :])
        nc.scalar.dma_start(
            out=wpb.rearrange("p c x -> p (c x)"), in_=wp[:, :]
        )

        # ---- window / center access patterns ----------------------------
        def win_all(ci, k):
            di, dj = divmod(k, KS)
            off = G + (ROWS * ci + di - 1) * SW + (dj - 1)
            return xg[:, :, off : off + ROWS * SW].rearrange(
                "p c (r w) -> p c r w", w=SW)[:, :, :, 0:W]

        def win_ct(ci, k, ct):
            di, dj = divmod(k, KS)
            off = G + (ROWS * ci + di - 1) * SW + (dj - 1)
            return xg[:, ct, off : off + ROWS * SW].rearrange(
                "p (r w) -> p r w", w=SW)[:, :, 0:W]

        def center(ci, kt, nh):
            off = G + (ROWS * ci + 8 * nh) * SW
            return xg[:, kt, off : off + 8 * SW].rearrange(
                "p (r w) -> p r w", w=SW)[:, :, 0:W]

        # ---- ECA pooling -------------------------------------------------
        # pieces 0,1 on DVE (head slack), 2,3 on scalar accum (hooked)
        def pool_dve(ci):
            lo = G + ROWS * ci * SW
            nc.vector.tensor_reduce(
                out=pacc[:, :, ci : ci + 1],
                in_=xg[:, :, lo : lo + ROWS * SW],
                axis=mybir.AxisListType.X,
                op=ALU.add,
            )

        def pool_scalar(ci):
            lo = G + ROWS * ci * SW
            for ct in range(CT):
                nc.scalar.activation(
                    out=pscr[:], in_=xg[:, ct, lo : lo + ROWS * SW],
                    func=AF.Copy, accum_out=pacc[:, ct, ci : ci + 1],
                )



        pool2 = consts.tile([128, CT], F32, tag="pool2", name="pool2")
        shd = consts.tile([128, CT], F32, tag="shd", name="shd")
        shu = consts.tile([128, CT], F32, tag="shu", name="shu")
        eca1 = consts.tile([128, CT], F32, tag="eca1", name="eca1")
        eca2 = consts.tile([128, CT], F32, tag="eca2", name="eca2")
        attn = consts.tile([128, CT], F32, tag="attn", name="attn")

        def emit_eca_combine():
            nc.gpsimd.tensor_tensor(
                out=pool2[:], in0=pacc[:, :, 0], in1=pacc[:, :, 1], op=ALU.add
            )
            nc.gpsimd.tensor_tensor(
                out=pool2[:], in0=pool2[:], in1=pacc[:, :, 2], op=ALU.add
            )
            nc.gpsimd.tensor_tensor(
                out=pool2[:], in0=pool2[:], in1=pacc[:, :, 3], op=ALU.add
            )
            nc.gpsimd.memset(shd[:], 0.0)
            nc.gpsimd.memset(shu[:], 0.0)
            for ct in range(CT):
                nc.gpsimd.dma_start(
                    out=shd[1:128, ct : ct + 1], in_=pool2[0:127, ct : ct + 1]
                )
                nc.gpsimd.dma_start(
                    out=shu[0:127, ct : ct + 1], in_=pool2[1:128, ct : ct + 1]
                )
            nc.gpsimd.dma_start(out=shd[0:1, 1:2], in_=pool2[127:128, 0:1])
            nc.gpsimd.dma_start(out=shu[127:128, 0:1], in_=pool2[0:1, 1:2])
            nc.vector.tensor_scalar(
                out=eca1, in0=shd[:], scalar1=wecab[:, 0:1], scalar2=None,
                op0=ALU.mult,
            )
            nc.vector.scalar_tensor_tensor(
                out=eca2, in0=pool2[:], scalar=wecab[:, 1:2], in1=eca1[:],
                op0=ALU.mult, op1=ALU.add,
            )
            nc.vector.scalar_tensor_tensor(
                out=eca1, in0=shu[:], scalar=wecab[:, 2:3], in1=eca2[:],
                op0=ALU.mult, op1=ALU.add,
            )

        # ---- main loop ---------------------------------------------------
        fused_t = [None] * NCHUNKS
        ypt_t = [None] * NCHUNKS
        pr_t = {}

        def emit_mm1_tap(ci, k):
            """mm1 for tap k (both ct out-tiles) + its evict/product."""
            dve_stt = k in DVE_STT_TAPS
            pr = prod_pool.tile([128, CT, CHUNK], BF16, tag=f"pr{k}",
                                name=f"pr{k}")
            fsb = None
            if not dve_stt:
                fsb = fsb_pool.tile([128, CT, CHUNK], BF16, tag="fsb",
                                    name="fsb")
            for ct in range(CT):
                mt = k * CT + ct
                fp = fps.tile([128, CHUNK], F32, tag="fp", name="fp")
                for kt in range(CT):
                    lhsT = wf_sb[kt][:, mt * 128 : (mt + 1) * 128]
                    for nh in range(NH):
                        nc.tensor.matmul(
                            fp[:, nh * 512 : (nh + 1) * 512],
                            lhsT,
                            center(ci, kt, nh),
                            start=(kt == 0),
                            stop=(kt == CT - 1),
                        )
                if dve_stt:
                    nc.vector.scalar_tensor_tensor(
                        out=pr[:, ct, :].rearrange("p (r w) -> p r w", w=W),
                        in0=fp[:].rearrange("p (r w) -> p r w", w=W),
                        scalar=bfp_sb[:, mt : mt + 1],
                        in1=win_ct(ci, k, ct),
                        op0=ALU.add, op1=ALU.mult,
                    )
                else:
                    nc.scalar.activation(
                        out=fsb[:, ct, :], in_=fp[:], func=AF.Identity,
                        bias=bfp_sb[:, mt : mt + 1], scale=1.0,
                    )
            pr_t[k] = pr
            return fsb, pr

        def emit_prod(ci, k, fsb, pr):
            if k in POOL_TAPS:
                for ct in range(CT):
                    nc.gpsimd.tensor_tensor(
                        out=pr[:, ct, :].rearrange("p (r w) -> p r w", w=W),
                        in0=fsb[:, ct, :].rearrange("p (r w) -> p r w", w=W),
                        in1=win_ct(ci, k, ct),
                        op=ALU.mult,
                    )
            else:
                nc.vector.tensor_tensor(
                    out=pr[:].rearrange("p c (r w) -> p c r w", w=W),
                    in0=fsb[:].rearrange("p c (r w) -> p c r w", w=W),
                    in1=win_all(ci, k),
                    op=ALU.mult,
                )

        def addp(a, b):
            nc.vector.tensor_add(pr_t[a][:], pr_t[a][:], pr_t[b][:])

        def emit_mm2_part(ci, mt2, nh, xc_first=False):
            # xc_first: x-branch MMs first (they don't need the fused tile;
            # used for the last chunk where fused lands late)
            yp = yps.tile([128, 512], F32, tag="yp", name="yp")
            srcs = [0, 1] if not xc_first else [1, 0]
            for si, s in enumerate(srcs):
                for kt in range(CT):
                    if s == 0:
                        lhsT = wp_sb[kt][:, mt2 * 128 : (mt2 + 1) * 128]
                        rhs = fused_t[ci][:, kt, nh * 512 : (nh + 1) * 512]
                    else:
                        lhsT = weff[kt][:, mt2 * 128 : (mt2 + 1) * 128]
                        rhs = center(ci, kt, nh)
                    nc.tensor.matmul(
                        yp[:], lhsT, rhs,
                        start=(si == 0 and kt == 0),
                        stop=(si == 1 and kt == CT - 1),
                    )
            if ypt_t[ci] is None:
                ypt_t[ci] = [[None] * NH for _ in range(CT)]
            ypt_t[ci][mt2][nh] = yp

        def emit_yev(ci, mt2):
            r0 = ci * ROWS
            for nh in range(NH):
                src = ypt_t[ci][mt2][nh]
                dst = y_sb[mt2][:, r0 * W + nh * 512 : r0 * W + (nh + 1) * 512]
                nc.scalar.activation(out=dst, in_=src[:], func=AF.Copy)

        def emit_bn(ci, mt2):
            # stats from the freshly evicted bf16 y slices (frees PSUM
            # sooner than reading the mm2 PSUM tiles; bn_stats caps at 512)
            r0 = ci * ROWS
            for nh in range(NH):
                lo = r0 * W + nh * 512
                nc.vector.bn_stats(
                    out=stats_sb[mt2][:, ci * NH + nh, :],
                    in_=y_sb[mt2][:, lo : lo + 512],
                )

        def emit_weff():
            for kt in range(CT):
                nc.vector.tensor_scalar(
                    out=weff[kt][:], in0=wp_sb[kt][:],
                    scalar1=attn[:, kt : kt + 1], scalar2=None, op0=ALU.mult,
                )

        def emit_chunk(ci):
            cj = ci - 1
            first = ci == 0
            ft = prod_pool.tile([128, CT, CHUNK], BF16, tag="fused",
                                name="fused", bufs=2)
            for k in range(KS * KS):
                fsb, pr = emit_mm1_tap(ci, k)
                # hooks between mm1 and the DVE product
                if not first:
                    if k == 3:
                        emit_mm2_part(cj, 0, 0)
                    elif k == 4:
                        emit_mm2_part(cj, 0, 1)
                    elif k == 5:
                        emit_yev(cj, 0)
                        emit_bn(cj, 0)
                    elif k == 6:
                        emit_mm2_part(cj, 1, 0)
                    elif k == 7:
                        emit_mm2_part(cj, 1, 1)
                else:
                    if k == 5:
                        pool_scalar(2)
                    elif k == 7:
                        pool_dve(3)
                if fsb is not None:
                    emit_prod(ci, k, fsb, pr)
                # add tree woven between taps (in-place into pr tiles)
                if k == 1:
                    addp(0, 1)
                elif k == 3:
                    addp(2, 3)
                    addp(0, 2)
                elif k == 5:
                    addp(4, 5)
                elif k == 7:
                    addp(6, 7)
                    addp(4, 6)
                    addp(0, 4)
                elif k == 8:
                    nc.vector.tensor_add(ft[:], pr_t[0][:], pr_t[8][:])
            fused_t[ci] = ft
            if not first:
                emit_yev(cj, 1)
                emit_bn(cj, 1)
            else:
                emit_eca_combine()
                # sigmoid = 1/(1+exp(-x)) with Exp on scalar (same act table)
                nc.scalar.activation(out=eca2[:], in_=eca1[:], func=AF.Exp,
                                     bias=zb[:, 0:1], scale=-1.0)
                nc.vector.tensor_scalar(
                    out=attn, in0=eca2[:], scalar1=1.0, scalar2=None,
                    op0=ALU.add,
                )
                nc.vector.reciprocal(out=attn[:], in_=attn[:])
                emit_weff()

        pool_dve(0)
        pool_dve(1)
        for ci in range(NCHUNKS):
            emit_chunk(ci)
            if ci == 0:
                # collective warmup: emitted after chunk 0 so its barrier
                # trigger doesn't stall the tensor queue during the head
                nc.sync.dma_start(out=warm_in[:], in_=zb[:])
                nc.gpsimd.collective_compute(
                    "AllReduce", ALU.add, replica_groups=RG,
                    ins=[warm_in[:].opt()], outs=[warm_out[:].opt()],
                )

        # last chunk's mm2 + stats; aggregate each mt2 as soon as its
        # stats are complete so only the pack+dma remain after mm2(3,1,*)
        ps = consts.tile([128, CT, 2], F32, tag="ps", name="ps")

        def emit_aggr(mt2):
            mv = consts.tile([128, 2], F32, tag=f"mv{mt2}", name=f"mv{mt2}")
            nc.vector.bn_aggr(out=mv[:], in_=stats_sb[mt2][:])
            nc.vector.tensor_scalar(
                out=ps[:, mt2, 0:1], in0=mv[:, 0:1], scalar1=float(HW),
                scalar2=None, op0=ALU.mult,
            )
            nc.vector.scalar_tensor_tensor(
                out=ps[:, mt2, 1:2], in0=mv[:, 0:1], scalar=mv[:, 0:1],
                in1=mv[:, 1:2], op0=ALU.mult, op1=ALU.add,
            )
            nc.vector.tensor_scalar(
                out=ps[:, mt2, 1:2], in0=ps[:, mt2, 1:2], scalar1=float(HW),
                scalar2=None, op0=ALU.mult,
            )

        c3 = NCHUNKS - 1
        for mt2 in range(CT):
            for nh in range(NH):
                emit_mm2_part(c3, mt2, nh, xc_first=True)
            # stats straight from the mm2 PSUM tiles (no wait on yev)
            r0 = c3 * ROWS
            for nh in range(NH):
                nc.vector.bn_stats(
                    out=stats_sb[mt2][:, c3 * NH + nh, :],
                    in_=ypt_t[c3][mt2][nh][:],
                )
            emit_aggr(mt2)
            emit_yev(c3, mt2)

        ps_b = dram.tile([128, CT * 2], F32, tag="psb", name="psb")
        gs_b = dram.tile([128, CT * 2], F32, tag="gsb", name="gsb",
                         addr_space="Shared")
        nc.sync.dma_start(out=ps_b[:], in_=ps.rearrange("p m two -> p (m two)"))
        nc.gpsimd.collective_compute(
            "AllReduce", ALU.add, replica_groups=RG,
            ins=[ps_b[:].opt()], outs=[gs_b[:].opt()],
        )
        # preload the Sqrt act table while the collective runs; input is a
        # ps slice so the dependency-driven scheduler cannot hoist the
        # table switch ahead of the Identity/Copy evicts
        sqscr = consts.tile([128, 1], F32, tag="sqscr", name="sqscr")
        nc.scalar.activation(out=sqscr[:], in_=ps[:, 0, 0:1], func=AF.Sqrt,
                             bias=zb[:, 0:1], scale=1.0)
        gs = consts.tile([128, CT, 2], F32, tag="gs", name="gs")
        nc.sync.dma_start(out=gs.rearrange("p m two -> p (m two)"), in_=gs_b[:])

        # ---- normalize and write out ------------------------------------
        minv = 1.0 / float(B * HW)
        mg = consts.tile([128, CT], F32, tag="mg", name="mg")
        vg = consts.tile([128, CT], F32, tag="vg", name="vg")
        rr = consts.tile([128, CT], F32, tag="rr", name="rr")
        tt = consts.tile([128, CT], F32, tag="tt", name="tt")
        ac = consts.tile([128, CT], F32, tag="ac", name="ac")
        bc = consts.tile([128, CT], F32, tag="bc", name="bc")
        nc.vector.tensor_scalar(
            out=mg[:], in0=gs[:, :, 0], scalar1=minv, scalar2=None, op0=ALU.mult
        )
        nc.vector.tensor_scalar(
            out=vg[:], in0=gs[:, :, 1], scalar1=minv, scalar2=None, op0=ALU.mult
        )
        nc.vector.tensor_tensor(out=tt[:], in0=mg[:], in1=mg[:], op=ALU.mult)
        nc.vector.tensor_tensor(out=vg[:], in0=vg[:], in1=tt[:], op=ALU.subtract)
        nc.vector.tensor_scalar(
            out=vg[:], in0=vg[:], scalar1=1.0, scalar2=BN_EPS,
            op0=ALU.mult, op1=ALU.add,
        )
        nc.scalar.activation(out=tt[:], in_=vg[:], func=AF.Sqrt,
                             bias=zb[:, 0:1], scale=1.0)
        nc.vector.reciprocal(out=rr[:], in_=tt[:])
        nc.vector.tensor_tensor(out=ac[:], in0=rr[:], in1=gam_sb[:], op=ALU.mult)
        nc.vector.tensor_tensor(out=bc[:], in0=mg[:], in1=ac[:], op=ALU.mult)
        nc.vector.tensor_tensor(out=bc[:], in0=bet_sb[:], in1=bc[:], op=ALU.subtract)

        NSL = 4
        SL = HW // NSL
        idx = 0
        for si in range(NSL):
            for mt2 in range(CT):
                sl = slice(si * SL, (si + 1) * SL)
                nc.vector.tensor_scalar(
                    out=y_sb[mt2][:, sl], in0=y_sb[mt2][:, sl],
                    scalar1=ac[:, mt2 : mt2 + 1], scalar2=bc[:, mt2 : mt2 + 1],
                    op0=ALU.mult, op1=ALU.add,
                )
                eng = nc.sync if idx % 2 == 0 else nc.scalar
                eng.dma_start(out=yout[mt2, :, sl], in_=y_sb[mt2][:, sl])
                idx += 1


_NC = None


def _build_nc(debug=False):
    nc = bacc.Bacc(
        "TRN2", target_bir_lowering=False, debug=debug, num_devices=NCORES
    )
    with tile.TileContext(nc, num_cores=NCORES) as tc:
        _emit(tc)
    nc.compile()
    return nc


def _get_nc():
    global _NC
    if _NC is None:
        _NC = _build_nc()
    return _NC


def _prep_in_maps(x, W_filter, b_filter, w_eca, W_proj, gamma, beta):
    bf = ml_dtypes.bfloat16
    x = np.asarray(x, np.float32)
    W_filter = np.asarray(W_filter, np.float32)
    b_filter = np.asarray(b_filter, np.float32)
    w_eca = np.asarray(w_eca, np.float32)
    W_proj = np.asarray(W_proj, np.float32)
    gamma = np.asarray(gamma, np.float32)
    beta = np.asarray(beta, np.float32)

    # gutter layout: row r at G + r*SW, col SW-1 of each row stays zero
    buf = np.zeros((B, C, XB), np.float32)
    xr = x.reshape(B, C, H, W)
    for r in range(H):
        buf[:, :, G + r * SW : G + r * SW + W] = xr[:, :, r, :]
    xg_h = np.ascontiguousarray(buf.reshape(B, CT, 128, XB)).astype(bf)

    # permute mm1 weights: o' = k*256 + c  (original o = c*9 + k)
    wperm = W_filter.reshape(C, KS * KS, C).transpose(1, 0, 2).reshape(KS * KS * C, C)
    wf_h = np.ascontiguousarray(wperm.T.reshape(CT, 128, MT1 * 128)).astype(bf)
    bperm = b_filter.reshape(C, KS * KS).T.reshape(KS * KS * C)

    wp_h = np.ascontiguousarray(
        (0.5 * W_proj).T.reshape(CT, 128, C).transpose(1, 0, 2).reshape(128, CT * C)
    ).astype(bf)
    misc_h = np.zeros((128, MT1 + 7), np.float32)
    misc_h[:, 0:MT1] = bperm.reshape(MT1, 128).T
    misc_h[:, MT1 : MT1 + 3] = (w_eca / float(HW)).reshape(1, 3)
    misc_h[:, MT1 + 3 : MT1 + 5] = gamma.reshape(CT, 128).T
    misc_h[:, MT1 + 5 : MT1 + 7] = beta.reshape(CT, 128).T

    in_maps = []
    for i in range(B):
        m = {
            "xg": xg_h[i],
            "wf": wf_h,
            "misc": misc_h,
            "wp": wp_h,
        }
        in_maps.append(m)
    return in_maps


last_result = None


def kernel(x, W_filter, b_filter, w_eca, W_proj, b_proj, gamma, beta):
    """Full-input, full-output DDF module on 8 NeuronCores."""
    global last_result
    # b_proj is mathematically cancelled by the batch-norm; unused.
    in_maps = _prep_in_maps(x, W_filter, b_filter, w_eca, W_proj, gamma, beta)
    nc = _get_nc()
    trace = bool(int(os.environ.get("DDF_TRACE", "0")))
    res = run_bass_kernel_spmd(nc, in_maps, list(range(NCORES)), trace=trace)
    last_result = res
    out = np.stack(
        [
            np.asarray(res.results[i]["y"]).reshape(C, H, W).astype(np.float32)
            for i in range(B)
        ]
    )
    return out


# revision 38
# speedup vs baseline: 1.0440x; 1.0018x over previous
"""Trainium2 Bass kernel for the DDF (dynamic-filter + ECA + BN) module.

Data-parallel over batch B=8 across 8 NeuronCores (one image per core),
params replicated, sync-BN via a single small AllReduce.

Layout: channels on partitions (CT=2 channel-tiles of 128); x lives in ONE
SBUF buffer with a 65-elem row stride whose 65th column is a zero "gutter".
All nine 3x3 window shifts are then plain slices of that buffer (the gutter
supplies the zero-pad at the row edges), so no derived shifted copies, no
wrap fix-ups.

Per 16-row chunk, the 18 mm1 PSUM tiles are drained by three engines in
parallel so the PE never waits on a PSUM bank:
  - Scalar taps: ACT evict (+bias) to SBUF, DVE does the bf16 window product.
  - Pool taps:   gpsimd scalar_tensor_tensor does (psum+bias)*window in one op.
The 9 tap products are summed by an 8-add DVE tree ordered by readiness, and
mm2 contracts {fused, attn*x} through W_proj (x-branch via attention-scaled
weights).  mm2 of chunk i is interleaved into mm1 of chunk i+1.  BN stats are
taken from the mm2 PSUM tiles (DVE bn_stats), aggregated, and exchanged with
one 2KB AllReduce on the sync queue; the Sqrt act-table load is hidden under
the AllReduce.  y is kept in bf16 and normalized in place, then written out
on two DMA queues.
"""

import os

import numpy as np
import ml_dtypes

import concourse.bass as bass
import concourse.mybir as mybir
import concourse.tile as tile
from concourse import bacc
from concourse.bass_utils import run_bass_kernel_spmd

B, C, H, W = 8, 256, 64, 64
KS = 3
HW = H * W                    # 4096
SW = W + 1                    # row stride with zero gutter column
G = SW + 1                    # guard elems at each end (covers di,dj = -1,-1)
XB = G + H * SW + G           # 4292 per channel-tile
NCORES = 8
CT = 2                        # channel tiles of 128
MT1 = KS * KS * CT            # 18 mm1 output m-tiles
BN_EPS = 1e-5
F32 = mybir.dt.float32
BF16 = mybir.dt.bfloat16
ROWS = 16                     # rows per chunk
NCHUNKS = H // ROWS           # 4
CHUNK = ROWS * W              # 1024 pixels per chunk per channel-tile
NH = CHUNK // 512             # 512-px matmul groups per chunk

AF = mybir.ActivationFunctionType
ALU = mybir.AluOpType
RG = [list(range(NCORES))]

# Pool cannot read PSUM on TRN2, and bulk gpsimd tensor_tensor traffic was
# measured to slow concurrent DVE ops ~2x (SBUF contention), so the drain
# pipeline uses Scalar (all 18 ACT evicts+bias) + DVE (all products, adds,
# bn) only — measured balanced at ~23us/chunk each.
POOL_TAPS = ()
DVE_STT_TAPS = (8,)           # drain tap 8 on DVE to shorten the chunk-end
                              # evict->product->fused critical chain


def _emit(tc):
    nc = tc.nc

    xgp = nc.declare_dram_parameter("xg", [CT, 128, XB], BF16, isOutput=False)
    wf = nc.declare_dram_parameter("wf", [CT, 128, MT1 * 128], BF16, isOutput=False)
    # misc fp32 params packed: bfp[18] | weca[3] | gam[2] | bet[2]
    misc = nc.declare_dram_parameter("misc", [128, MT1 + 7], F32, isOutput=False)
    wp = nc.declare_dram_parameter("wp", [128, CT * C], BF16, isOutput=False)
    yout = nc.declare_dram_parameter("y", [CT, 128, HW], BF16, isOutput=True)

    with (
        tc.tile_pool(name="consts", bufs=1) as consts,
        tc.tile_pool(name="fps", bufs=3, space="PSUM") as fps,
        tc.tile_pool(name="yps", bufs=2, space="PSUM") as yps,
        tc.tile_pool(name="fsb", bufs=5) as fsb_pool,
        tc.tile_pool(name="prod", bufs=1) as prod_pool,
        tc.tile_pool(name="dram", bufs=1, space="DRAM") as dram,
    ):
        # ---- resident tensors -------------------------------------------
        wf_sb = [consts.tile([128, MT1 * 128], BF16, tag=f"wf{kt}", name=f"wf{kt}")
                 for kt in range(CT)]
        wpb = consts.tile([128, CT, C], BF16, tag="wpb", name="wpb")
        wp_sb = [wpb[:, kt, :] for kt in range(CT)]
        weffb = consts.tile([128, CT, C], BF16, tag="weffb", name="weffb")
        weff = [weffb[:, kt, :] for kt in range(CT)]
        miscb = consts.tile([128, MT1 + 7], F32, tag="miscb", name="miscb")
        bfp_sb = miscb[:, 0:MT1]
        wecab = miscb[:, MT1 : MT1 + 3]
        gam_sb = miscb[:, MT1 + 3 : MT1 + 5]
        bet_sb = miscb[:, MT1 + 5 : MT1 + 7]
        xg = consts.tile([128, CT, XB], BF16, tag="xg", name="xg")
        y_sb = [consts.tile([128, HW], BF16, tag=f"ysb{mt}", name=f"ysb{mt}")
                for mt in range(CT)]
        stats_sb = [
            consts.tile([128, NCHUNKS * NH, 6], F32, tag=f"st{mt}", name=f"st{mt}")
            for mt in range(CT)
        ]
        pscr = consts.tile([128, ROWS * SW], F32, tag="pscr", name="pscr")
        pacc = consts.tile([128, CT, NCHUNKS], F32, tag="pacc", name="pacc")
        zb = consts.tile([128, 1], F32, tag="zb", name="zb")
        nc.vector.memset(zb[:], 0.0)

        warm_in = dram.tile([128, 1], F32, tag="wi", name="wi")
        warm_out = dram.tile([128, 1], F32, tag="wo", name="wo",
                             addr_space="Shared")

        # ---- input DMAs --------------------------------------------------
        # x in 4 overlapping row pieces per ct (halo rows included so chunk
        # i's windows only read pieces <= i); ct0 on sync, ct1 on gpsimd.
        # wf tiles split scalar/sync so both land within ~2.5us.
        pieces = []
        for i in range(NCHUNKS):
            lo = 0 if i == 0 else G + (ROWS * i - 1) * SW - 2
            hi = XB if i == NCHUNKS - 1 else G + (ROWS * i + ROWS + 1) * SW
            pieces.append((lo, hi))
        nc.sync.dma_start(out=xg[:, 0, pieces[0][0]:pieces[0][1]],
                          in_=xgp[0, :, pieces[0][0]:pieces[0][1]])
        nc.gpsimd.dma_start(out=xg[:, 1, pieces[0][0]:pieces[0][1]],
                            in_=xgp[1, :, pieces[0][0]:pieces[0][1]])
        nc.scalar.dma_start(out=wf_sb[0][:], in_=wf[0])
        nc.sync.dma_start(out=wf_sb[1][:], in_=wf[1])
        for lo, hi in pieces[1:]:
            nc.sync.dma_start(out=xg[:, 0, lo:hi], in_=xgp[0, :, lo:hi])
            nc.gpsimd.dma_start(out=xg[:, 1, lo:hi], in_=xgp[1, :, lo:hi])
        nc.scalar.dma_start(out=miscb[:], in_=misc[:, :])
        nc.scalar.dma_start(
            out=wpb.rearrange("p c x -> p (c x)"), in_=wp[:, :]
        )

        # ---- window / center access patterns ----------------------------
        def win_all(ci, k):
            di, dj = divmod(k, KS)
            off = G + (ROWS * ci + di - 1) * SW + (dj - 1)
            return xg[:, :, off : off + ROWS * SW].rearrange(
                "p c (r w) -> p c r w", w=SW)[:, :, :, 0:W]

        def win_ct(ci, k, ct):
            di, dj = divmod(k, KS)
            off = G + (ROWS * ci + di - 1) * SW + (dj - 1)
            return xg[:, ct, off : off + ROWS * SW].rearrange(
                "p (r w) -> p r w", w=SW)[:, :, 0:W]

        def center(ci, kt, nh):
            off = G + (ROWS * ci + 8 * nh) * SW
            return xg[:, kt, off : off + 8 * SW].rearrange(
                "p (r w) -> p r w", w=SW)[:, :, 0:W]

        # ---- ECA pooling -------------------------------------------------
        # pieces 0,1 on DVE (head slack), 2,3 on scalar accum (hooked)
        def pool_dve(ci):
            lo = G + ROWS * ci * SW
            nc.vector.tensor_reduce(
                out=pacc[:, :, ci : ci + 1],
                in_=xg[:, :, lo : lo + ROWS * SW],
                axis=mybir.AxisListType.X,
                op=ALU.add,
            )

        def pool_scalar(ci):
            lo = G + ROWS * ci * SW
            for ct in range(CT):
                nc.scalar.activation(
                    out=pscr[:], in_=xg[:, ct, lo : lo + ROWS * SW],
                    func=AF.Copy, accum_out=pacc[:, ct, ci : ci + 1],
                )



        pool2 = consts.tile([128, CT], F32, tag="pool2", name="pool2")
        shd = consts.tile([128, CT], F32, tag="shd", name="shd")
        shu = consts.tile([128, CT], F32, tag="shu", name="shu")
        eca1 = consts.tile([128, CT], F32, tag="eca1", name="eca1")
        eca2 = consts.tile([128, CT], F32, tag="eca2", name="eca2")
        attn = consts.tile([128, CT], F32, tag="attn", name="attn")

        def emit_eca_combine():
            nc.gpsimd.tensor_tensor(
                out=pool2[:], in0=pacc[:, :, 0], in1=pacc[:, :, 1], op=ALU.add
            )
            nc.gpsimd.tensor_tensor(
                out=pool2[:], in0=pool2[:], in1=pacc[:, :, 2], op=ALU.add
            )
            nc.gpsimd.tensor_tensor(
                out=pool2[:], in0=pool2[:], in1=pacc[:, :, 3], op=ALU.add
            )
            nc.gpsimd.memset(shd[:], 0.0)
            nc.gpsimd.memset(shu[:], 0.0)
            for ct in range(CT):
                nc.gpsimd.dma_start(
                    out=shd[1:128, ct : ct + 1], in_=pool2[0:127, ct : ct + 1]
                )
                nc.gpsimd.dma_start(
                    out=shu[0:127, ct : ct + 1], in_=pool2[1:128, ct : ct + 1]
                )
            nc.gpsimd.dma_start(out=shd[0:1, 1:2], in_=pool2[127:128, 0:1])
            nc.gpsimd.dma_start(out=shu[127:128, 0:1], in_=pool2[0:1, 1:2])
            nc.vector.tensor_scalar(
                out=eca1, in0=shd[:], scalar1=wecab[:, 0:1], scalar2=None,
                op0=ALU.mult,
            )
            nc.vector.scalar_tensor_tensor(
                out=eca2, in0=pool2[:], scalar=wecab[:, 1:2], in1=eca1[:],
                op0=ALU.mult, op1=ALU.add,
            )
            nc.vector.scalar_tensor_tensor(
                out=eca1, in0=shu[:], scalar=wecab[:, 2:3], in1=eca2[:],
                op0=ALU.mult, op1=ALU.add,
            )

        # ---- main loop ---------------------------------------------------
        fused_t = [None] * NCHUNKS
        ypt_t = [None] * NCHUNKS
        pr_t = {}

        def emit_mm1_tap(ci, k):
            """mm1 for tap k (both ct out-tiles) + its evict/product."""
            dve_stt = k in DVE_STT_TAPS
            pr = prod_pool.tile([128, CT, CHUNK], BF16, tag=f"pr{k}",
                                name=f"pr{k}")
            fsb = None
            if not dve_stt:
                fsb = fsb_pool.tile([128, CT, CHUNK], BF16, tag="fsb",
                                    name="fsb")
            for ct in range(CT):
                mt = k * CT + ct
                fp = fps.tile([128, CHUNK], F32, tag="fp", name="fp")
                for kt in range(CT):
                    lhsT = wf_sb[kt][:, mt * 128 : (mt + 1) * 128]
                    for nh in range(NH):
                        nc.tensor.matmul(
                            fp[:, nh * 512 : (nh + 1) * 512],
                            lhsT,
                            center(ci, kt, nh),
                            start=(kt == 0),
                            stop=(kt == CT - 1),
                        )
                if dve_stt:
                    nc.vector.scalar_tensor_tensor(
                        out=pr[:, ct, :].rearrange("p (r w) -> p r w", w=W),
                        in0=fp[:].rearrange("p (r w) -> p r w", w=W),
                        scalar=bfp_sb[:, mt : mt + 1],
                        in1=win_ct(ci, k, ct),
                        op0=ALU.add, op1=ALU.mult,
                    )
                else:
                    nc.scalar.activation(
                        out=fsb[:, ct, :], in_=fp[:], func=AF.Identity,
                        bias=bfp_sb[:, mt : mt + 1], scale=1.0,
                    )
            pr_t[k] = pr
            return fsb, pr

        def emit_prod(ci, k, fsb, pr):
            if k in POOL_TAPS:
                for ct in range(CT):
                    nc.gpsimd.tensor_tensor(
                        out=pr[:, ct, :].rearrange("p (r w) -> p r w", w=W),
                        in0=fsb[:, ct, :].rearrange("p (r w) -> p r w", w=W),
                        in1=win_ct(ci, k, ct),
                        op=ALU.mult,
                    )
            else:
                nc.vector.tensor_tensor(
                    out=pr[:].rearrange("p c (r w) -> p c r w", w=W),
                    in0=fsb[:].rearrange("p c (r w) -> p c r w", w=W),
                    in1=win_all(ci, k),
                    op=ALU.mult,
                )

        def addp(a, b):
            nc.vector.tensor_add(pr_t[a][:], pr_t[a][:], pr_t[b][:])

        def emit_mm2_part(ci, mt2, nh, xc_first=False):
            # xc_first: x-branch MMs first (they don't need the fused tile;
            # used for the last chunk where fused lands late)
            yp = yps.tile([128, 512], F32, tag="yp", name="yp")
            srcs = [0, 1] if not xc_first else [1, 0]
            for si, s in enumerate(srcs):
                for kt in range(CT):
                    if s == 0:
                        lhsT = wp_sb[kt][:, mt2 * 128 : (mt2 + 1) * 128]
                        rhs = fused_t[ci][:, kt, nh * 512 : (nh + 1) * 512]
                    else:
                        lhsT = weff[kt][:, mt2 * 128 : (mt2 + 1) * 128]
                        rhs = center(ci, kt, nh)
                    nc.tensor.matmul(
                        yp[:], lhsT, rhs,
                        start=(si == 0 and kt == 0),
                        stop=(si == 1 and kt == CT - 1),
                    )
            if ypt_t[ci] is None:
                ypt_t[ci] = [[None] * NH for _ in range(CT)]
            ypt_t[ci][mt2][nh] = yp

        def emit_yev(ci, mt2):
            r0 = ci * ROWS
            for nh in range(NH):
                src = ypt_t[ci][mt2][nh]
                dst = y_sb[mt2][:, r0 * W + nh * 512 : r0 * W + (nh + 1) * 512]
                nc.scalar.activation(out=dst, in_=src[:], func=AF.Copy)

        def emit_bn(ci, mt2):
            # stats from the freshly evicted bf16 y slices (frees PSUM
            # sooner than reading the mm2 PSUM tiles; bn_stats caps at 512)
            r0 = ci * ROWS
            for nh in range(NH):
                lo = r0 * W + nh * 512
                nc.vector.bn_stats(
                    out=stats_sb[mt2][:, ci * NH + nh, :],
                    in_=y_sb[mt2][:, lo : lo + 512],
                )

        def emit_weff():
            for kt in range(CT):
                nc.vector.tensor_scalar(
                    out=weff[kt][:], in0=wp_sb[kt][:],
                    scalar1=attn[:, kt : kt + 1], scalar2=None, op0=ALU.mult,
                )

        def emit_chunk(ci):
            cj = ci - 1
            first = ci == 0
            ft = prod_pool.tile([128, CT, CHUNK], BF16, tag="fused",
                                name="fused", bufs=2)
            for k in range(KS * KS):
                fsb, pr = emit_mm1_tap(ci, k)
                # hooks between mm1 and the DVE product
                if not first:
                    if k == 3:
                        emit_mm2_part(cj, 0, 0)
                    elif k == 4:
                        emit_mm2_part(cj, 0, 1)
                    elif k == 5:
                        emit_yev(cj, 0)
                        emit_bn(cj, 0)
                    elif k == 6:
                        emit_mm2_part(cj, 1, 0)
                    elif k == 7:
                        emit_mm2_part(cj, 1, 1)
                else:
                    if k == 5:
                        pool_scalar(2)
                    elif k == 7:
                        pool_dve(3)
                if fsb is not None:
                    emit_prod(ci, k, fsb, pr)
                # add tree woven between taps (in-place into pr tiles)
                if k == 1:
                    addp(0, 1)
                elif k == 3:
                    addp(2, 3)
                    addp(0, 2)
                elif k == 5:
                    addp(4, 5)
                elif k == 7:
                    addp(6, 7)
                    addp(4, 6)
                    addp(0, 4)
                elif k == 8:
                    nc.vector.tensor_add(ft[:], pr_t[0][:], pr_t[8][:])
            fused_t[ci] = ft
            if not first:
                emit_yev(cj, 1)
                emit_bn(cj, 1)
            else:
                emit_eca_combine()
                # sigmoid = 1/(1+exp(-x)) with Exp on scalar (same act table)
                nc.scalar.activation(out=eca2[:], in_=eca1[:], func=AF.Exp,
                                     bias=zb[:, 0:1], scale=-1.0)
                nc.vector.tensor_scalar(
                    out=attn, in0=eca2[:], scalar1=1.0, scalar2=None,
                    op0=ALU.add,
                )
                nc.vector.reciprocal(out=attn[:], in_=attn[:])
                emit_weff()

        pool_dve(0)
        pool_dve(1)
        for ci in range(NCHUNKS):
            emit_chunk(ci)
            if ci == 0:
                # collective warmup: emitted after chunk 0 so its barrier
                # trigger doesn't stall the tensor queue during the head
                nc.sync.dma_start(out=warm_in[:], in_=zb[:])
                nc.gpsimd.collective_compute(
                    "AllReduce", ALU.add, replica_groups=RG,
                    ins=[warm_in[:].opt()], outs=[warm_out[:].opt()],
                )

        # last chunk's mm2 + stats; aggregate each mt2 as soon as its
        # stats are complete so only the pack+dma remain after mm2(3,1,*)
        ps = consts.tile([128, CT, 2], F32, tag="ps", name="ps")

        def emit_aggr(mt2):
            mv = consts.tile([128, 2], F32, tag=f"mv{mt2}", name=f"mv{mt2}")
            nc.vector.bn_aggr(out=mv[:], in_=stats_sb[mt2][:])
            nc.vector.tensor_scalar(
                out=ps[:, mt2, 0:1], in0=mv[:, 0:1], scalar1=float(HW),
                scalar2=None, op0=ALU.mult,
            )
            nc.vector.scalar_tensor_tensor(
                out=ps[:, mt2, 1:2], in0=mv[:, 0:1], scalar=mv[:, 0:1],
                in1=mv[:, 1:2], op0=ALU.mult, op1=ALU.add,
            )
            nc.vector.tensor_scalar(
                out=ps[:, mt2, 1:2], in0=ps[:, mt2, 1:2], scalar1=float(HW),
                scalar2=None, op0=ALU.mult,
            )

        c3 = NCHUNKS - 1
        for mt2 in range(CT):
            for nh in range(NH):
                emit_mm2_part(c3, mt2, nh, xc_first=True)
            # stats straight from the mm2 PSUM tiles (no wait on yev)
            r0 = c3 * ROWS
            for nh in range(NH):
                nc.vector.bn_stats(
                    out=stats_sb[mt2][:, c3 * NH + nh, :],
                    in_=ypt_t[c3][mt2][nh][:],
                )
            emit_aggr(mt2)
            emit_yev(c3, mt2)

        # exchange the per-channel sums in bf16 (halves the collective
        # payload; ~0.4% stats error, well inside the accuracy budget)
        psh = consts.tile([128, CT * 2], BF16, tag="psh", name="psh")
        nc.vector.tensor_scalar(
            out=psh[:], in0=ps.rearrange("p m two -> p (m two)"),
            scalar1=1.0, scalar2=None, op0=ALU.mult,
        )
        ps_b = dram.tile([128, CT * 2], BF16, tag="psb", name="psb")
        gs_b = dram.tile([128, CT * 2], BF16, tag="gsb", name="gsb",
                         addr_space="Shared")
        nc.sync.dma_start(out=ps_b[:], in_=psh[:])
        nc.gpsimd.collective_compute(
            "AllReduce", ALU.add, replica_groups=RG,
            ins=[ps_b[:].opt()], outs=[gs_b[:].opt()],
        )
        # preload the Sqrt act table while the collective runs; input is a
        # ps slice so the dependency-driven scheduler cannot hoist the
        # table switch ahead of the Identity/Copy evicts
        sqscr = consts.tile([128, 1], F32, tag="sqscr", name="sqscr")
        nc.scalar.activation(out=sqscr[:], in_=ps[:, 0, 0:1], func=AF.Sqrt,
                             bias=zb[:, 0:1], scale=1.0)
        gs = consts.tile([128, CT, 2], BF16, tag="gs", name="gs")
        nc.sync.dma_start(out=gs.rearrange("p m two -> p (m two)"), in_=gs_b[:])

        # ---- normalize and write out ------------------------------------
        minv = 1.0 / float(B * HW)
        mg = consts.tile([128, CT], F32, tag="mg", name="mg")
        vg = consts.tile([128, CT], F32, tag="vg", name="vg")
        rr = consts.tile([128, CT], F32, tag="rr", name="rr")
        tt = consts.tile([128, CT], F32, tag="tt", name="tt")
        ac = consts.tile([128, CT], F32, tag="ac", name="ac")
        bc = consts.tile([128, CT], F32, tag="bc", name="bc")
        nc.vector.tensor_scalar(
            out=mg[:], in0=gs[:, :, 0], scalar1=minv, scalar2=None, op0=ALU.mult
        )
        nc.vector.tensor_scalar(
            out=vg[:], in0=gs[:, :, 1], scalar1=minv, scalar2=None, op0=ALU.mult
        )
        nc.vector.tensor_tensor(out=tt[:], in0=mg[:], in1=mg[:], op=ALU.mult)
        nc.vector.tensor_tensor(out=vg[:], in0=vg[:], in1=tt[:], op=ALU.subtract)
        nc.vector.tensor_scalar(
            out=vg[:], in0=vg[:], scalar1=1.0, scalar2=BN_EPS,
            op0=ALU.mult, op1=ALU.add,
        )
        nc.scalar.activation(out=tt[:], in_=vg[:], func=AF.Sqrt,
                             bias=zb[:, 0:1], scale=1.0)
        nc.vector.reciprocal(out=rr[:], in_=tt[:])
        nc.vector.tensor_tensor(out=ac[:], in0=rr[:], in1=gam_sb[:], op=ALU.mult)
        nc.vector.tensor_tensor(out=bc[:], in0=mg[:], in1=ac[:], op=ALU.mult)
        nc.vector.tensor_tensor(out=bc[:], in0=bet_sb[:], in1=bc[:], op=ALU.subtract)

        NSL = 4
        SL = HW // NSL
        idx = 0
        for si in range(NSL):
            for mt2 in range(CT):
                sl = slice(si * SL, (si + 1) * SL)
                nc.vector.tensor_scalar(
                    out=y_sb[mt2][:, sl], in0=y_sb[mt2][:, sl],
                    scalar1=ac[:, mt2 : mt2 + 1], scalar2=bc[:, mt2 : mt2 + 1],
                    op0=ALU.mult, op1=ALU.add,
                )
                eng = nc.sync if idx % 2 == 0 else nc.scalar
                eng.dma_start(out=yout[mt2, :, sl], in_=y_sb[mt2][:, sl])
                idx += 1


_NC = None


def _build_nc(debug=False):
    nc = bacc.Bacc(
        "TRN2", target_bir_lowering=False, debug=debug, num_devices=NCORES
    )
    with tile.TileContext(nc, num_cores=NCORES) as tc:
        _emit(tc)
    nc.compile()
    return nc


def _get_nc():
    global _NC
    if _NC is None:
        _NC = _build_nc()
    return _NC


def _prep_in_maps(x, W_filter, b_filter, w_eca, W_proj, gamma, beta):
    bf = ml_dtypes.bfloat16
    x = np.asarray(x, np.float32)
    W_filter = np.asarray(W_filter, np.float32)
    b_filter = np.asarray(b_filter, np.float32)
    w_eca = np.asarray(w_eca, np.float32)
    W_proj = np.asarray(W_proj, np.float32)
    gamma = np.asarray(gamma, np.float32)
    beta = np.asarray(beta, np.float32)

    # gutter layout: row r at G + r*SW, col SW-1 of each row stays zero
    buf = np.zeros((B, C, XB), np.float32)
    xr = x.reshape(B, C, H, W)
    for r in range(H):
        buf[:, :, G + r * SW : G + r * SW + W] = xr[:, :, r, :]
    xg_h = np.ascontiguousarray(buf.reshape(B, CT, 128, XB)).astype(bf)

    # permute mm1 weights: o' = k*256 + c  (original o = c*9 + k)
    wperm = W_filter.reshape(C, KS * KS, C).transpose(1, 0, 2).reshape(KS * KS * C, C)
    wf_h = np.ascontiguousarray(wperm.T.reshape(CT, 128, MT1 * 128)).astype(bf)
    bperm = b_filter.reshape(C, KS * KS).T.reshape(KS * KS * C)

    wp_h = np.ascontiguousarray(
        (0.5 * W_proj).T.reshape(CT, 128, C).transpose(1, 0, 2).reshape(128, CT * C)
    ).astype(bf)
    misc_h = np.zeros((128, MT1 + 7), np.float32)
    misc_h[:, 0:MT1] = bperm.reshape(MT1, 128).T
    misc_h[:, MT1 : MT1 + 3] = (w_eca / float(HW)).reshape(1, 3)
    misc_h[:, MT1 + 3 : MT1 + 5] = gamma.reshape(CT, 128).T
    misc_h[:, MT1 + 5 : MT1 + 7] = beta.reshape(CT, 128).T

    in_maps = []
    for i in range(B):
        m = {
            "xg": xg_h[i],
            "wf": wf_h,
            "misc": misc_h,
            "wp": wp_h,
        }
        in_maps.append(m)
    return in_maps


last_result = None


def kernel(x, W_filter, b_filter, w_eca, W_proj, b_proj, gamma, beta):
    """Full-input, full-output DDF module on 8 NeuronCores."""
    global last_result
    # b_proj is mathematically cancelled by the batch-norm; unused.
    in_maps = _prep_in_maps(x, W_filter, b_filter, w_eca, W_proj, gamma, beta)
    nc = _get_nc()
    trace = bool(int(os.environ.get("DDF_TRACE", "0")))
    res = run_bass_kernel_spmd(nc, in_maps, list(range(NCORES)), trace=trace)
    last_result = res
    out = np.stack(
        [
            np.asarray(res.results[i]["y"]).reshape(C, H, W).astype(np.float32)
            for i in range(B)
        ]
    )
    return out


# revision 42
# speedup vs baseline: 1.1321x; 1.0844x over previous
"""Trainium2 Bass kernel for the DDF (dynamic-filter + ECA + BN) module.

Data-parallel over batch B=8 across 8 NeuronCores (one image per core),
params replicated, sync-BN via a single small AllReduce.

Layout: channels on partitions (CT=2 channel-tiles of 128); x lives in ONE
SBUF buffer with a 65-elem row stride whose 65th column is a zero "gutter".
All nine 3x3 window shifts are then plain slices of that buffer (the gutter
supplies the zero-pad at the row edges), so no derived shifted copies, no
wrap fix-ups.

Per 16-row chunk, the 18 mm1 PSUM tiles are drained by three engines in
parallel so the PE never waits on a PSUM bank:
  - Scalar taps: ACT evict (+bias) to SBUF, DVE does the bf16 window product.
  - Pool taps:   gpsimd scalar_tensor_tensor does (psum+bias)*window in one op.
The 9 tap products are summed by an 8-add DVE tree ordered by readiness, and
mm2 contracts {fused, attn*x} through W_proj (x-branch via attention-scaled
weights).  mm2 of chunk i is interleaved into mm1 of chunk i+1.  BN stats are
taken from the mm2 PSUM tiles (DVE bn_stats), aggregated, and exchanged with
one 2KB AllReduce on the sync queue; the Sqrt act-table load is hidden under
the AllReduce.  y is kept in bf16 and normalized in place, then written out
on two DMA queues.
"""

import os

import numpy as np
import ml_dtypes

import concourse.bass as bass
import concourse.mybir as mybir
import concourse.tile as tile
from concourse import bacc
from concourse.bass_utils import run_bass_kernel_spmd

B, C, H, W = 8, 256, 64, 64
KS = 3
HW = H * W                    # 4096
SW = W + 1                    # row stride with zero gutter column
G = SW + 1                    # guard elems at each end (covers di,dj = -1,-1)
XB = G + H * SW + G           # 4292 per channel-tile
NCORES = 8
CT = 2                        # channel tiles of 128
MT1 = KS * KS * CT            # 18 mm1 output m-tiles
BN_EPS = 1e-5
F32 = mybir.dt.float32
BF16 = mybir.dt.bfloat16
ROWS = 16                     # rows per chunk
NCHUNKS = H // ROWS           # 4
CHUNK = ROWS * W              # 1024 pixels per chunk per channel-tile
NH = CHUNK // 512             # 512-px matmul groups per chunk

AF = mybir.ActivationFunctionType
ALU = mybir.AluOpType
RG = [list(range(NCORES))]

# Pool cannot read PSUM on TRN2, and bulk gpsimd tensor_tensor traffic was
# measured to slow concurrent DVE ops ~2x (SBUF contention), so the drain
# pipeline uses Scalar (all 18 ACT evicts+bias) + DVE (all products, adds,
# bn) only — measured balanced at ~23us/chunk each.
POOL_TAPS = ()
DVE_STT_TAPS = (8,)           # drain tap 8 on DVE to shorten the chunk-end
                              # evict->product->fused critical chain


def _emit(tc):
    nc = tc.nc

    xgp = nc.declare_dram_parameter("xg", [CT, 128, XB], BF16, isOutput=False)
    wf = nc.declare_dram_parameter("wf", [CT, 128, MT1 * 128], BF16, isOutput=False)
    # misc fp32 params packed: bfp[18] | weca[3] | gam[2] | bet[2]
    misc = nc.declare_dram_parameter("misc", [128, MT1 + 7], F32, isOutput=False)
    wp = nc.declare_dram_parameter("wp", [128, CT * C], BF16, isOutput=False)
    yout = nc.declare_dram_parameter("y", [CT, 128, HW], BF16, isOutput=True)

    with (
        tc.tile_pool(name="consts", bufs=1) as consts,
        tc.tile_pool(name="fps", bufs=3, space="PSUM") as fps,
        tc.tile_pool(name="yps", bufs=2, space="PSUM") as yps,
        tc.tile_pool(name="fsb", bufs=5) as fsb_pool,
        tc.tile_pool(name="prod", bufs=1) as prod_pool,
        tc.tile_pool(name="dram", bufs=1, space="DRAM") as dram,
    ):
        # ---- resident tensors -------------------------------------------
        wf_sb = [consts.tile([128, MT1 * 128], BF16, tag=f"wf{kt}", name=f"wf{kt}")
                 for kt in range(CT)]
        wpb = consts.tile([128, CT, C], BF16, tag="wpb", name="wpb")
        wp_sb = [wpb[:, kt, :] for kt in range(CT)]
        weffb = consts.tile([128, CT, C], BF16, tag="weffb", name="weffb")
        weff = [weffb[:, kt, :] for kt in range(CT)]
        miscb = consts.tile([128, MT1 + 7], F32, tag="miscb", name="miscb")
        bfp_sb = miscb[:, 0:MT1]
        wecab = miscb[:, MT1 : MT1 + 3]
        gam_sb = miscb[:, MT1 + 3 : MT1 + 5]
        bet_sb = miscb[:, MT1 + 5 : MT1 + 7]
        xg = consts.tile([128, CT, XB], BF16, tag="xg", name="xg")
        y_sb = [consts.tile([128, HW], BF16, tag=f"ysb{mt}", name=f"ysb{mt}")
                for mt in range(CT)]
        stats_sb = [
            consts.tile([128, NCHUNKS * NH, 6], F32, tag=f"st{mt}", name=f"st{mt}")
            for mt in range(CT)
        ]
        pscr = consts.tile([128, ROWS * SW], F32, tag="pscr", name="pscr")
        pacc = consts.tile([128, CT, NCHUNKS], F32, tag="pacc", name="pacc")
        zb = consts.tile([128, 1], F32, tag="zb", name="zb")
        nc.vector.memset(zb[:], 0.0)

        warm_in = dram.tile([128, 1], F32, tag="wi", name="wi")
        warm_out = dram.tile([128, 1], F32, tag="wo", name="wo",
                             addr_space="Shared")

        # ---- input DMAs --------------------------------------------------
        # x in 4 overlapping row pieces per ct (halo rows included so chunk
        # i's windows only read pieces <= i); ct0 on sync, ct1 on gpsimd.
        # wf tiles split scalar/sync so both land within ~2.5us.
        pieces = []
        for i in range(NCHUNKS):
            lo = 0 if i == 0 else G + (ROWS * i - 1) * SW - 2
            hi = XB if i == NCHUNKS - 1 else G + (ROWS * i + ROWS + 1) * SW
            pieces.append((lo, hi))
        nc.sync.dma_start(out=xg[:, 0, pieces[0][0]:pieces[0][1]],
                          in_=xgp[0, :, pieces[0][0]:pieces[0][1]])
        nc.gpsimd.dma_start(out=xg[:, 1, pieces[0][0]:pieces[0][1]],
                            in_=xgp[1, :, pieces[0][0]:pieces[0][1]])
        nc.scalar.dma_start(out=wf_sb[0][:], in_=wf[0])
        nc.sync.dma_start(out=wf_sb[1][:], in_=wf[1])
        for lo, hi in pieces[1:]:
            nc.sync.dma_start(out=xg[:, 0, lo:hi], in_=xgp[0, :, lo:hi])
            nc.gpsimd.dma_start(out=xg[:, 1, lo:hi], in_=xgp[1, :, lo:hi])
        nc.scalar.dma_start(out=miscb[:], in_=misc[:, :])
        nc.scalar.dma_start(
            out=wpb.rearrange("p c x -> p (c x)"), in_=wp[:, :]
        )

        # ---- window / center access patterns ----------------------------
        def win_all(ci, k):
            di, dj = divmod(k, KS)
            off = G + (ROWS * ci + di - 1) * SW + (dj - 1)
            return xg[:, :, off : off + ROWS * SW].rearrange(
                "p c (r w) -> p c r w", w=SW)[:, :, :, 0:W]

        def win_ct(ci, k, ct):
            di, dj = divmod(k, KS)
            off = G + (ROWS * ci + di - 1) * SW + (dj - 1)
            return xg[:, ct, off : off + ROWS * SW].rearrange(
                "p (r w) -> p r w", w=SW)[:, :, 0:W]

        def center(ci, kt, nh):
            off = G + (ROWS * ci + 8 * nh) * SW
            return xg[:, kt, off : off + 8 * SW].rearrange(
                "p (r w) -> p r w", w=SW)[:, :, 0:W]

        # ---- ECA pooling -------------------------------------------------
        # pieces 0,1 on DVE (head slack), 2,3 on scalar accum (hooked)
        def pool_dve(ci):
            lo = G + ROWS * ci * SW
            nc.vector.tensor_reduce(
                out=pacc[:, :, ci : ci + 1],
                in_=xg[:, :, lo : lo + ROWS * SW],
                axis=mybir.AxisListType.X,
                op=ALU.add,
            )

        def pool_scalar(ci):
            lo = G + ROWS * ci * SW
            for ct in range(CT):
                nc.scalar.activation(
                    out=pscr[:], in_=xg[:, ct, lo : lo + ROWS * SW],
                    func=AF.Copy, accum_out=pacc[:, ct, ci : ci + 1],
                )



        pool2 = consts.tile([128, CT], F32, tag="pool2", name="pool2")
        shd = consts.tile([128, CT], F32, tag="shd", name="shd")
        shu = consts.tile([128, CT], F32, tag="shu", name="shu")
        eca1 = consts.tile([128, CT], F32, tag="eca1", name="eca1")
        eca2 = consts.tile([128, CT], F32, tag="eca2", name="eca2")
        attn = consts.tile([128, CT], F32, tag="attn", name="attn")

        def emit_eca_combine():
            nc.gpsimd.tensor_tensor(
                out=pool2[:], in0=pacc[:, :, 0], in1=pacc[:, :, 1], op=ALU.add
            )
            nc.gpsimd.tensor_tensor(
                out=pool2[:], in0=pool2[:], in1=pacc[:, :, 2], op=ALU.add
            )
            nc.gpsimd.tensor_tensor(
                out=pool2[:], in0=pool2[:], in1=pacc[:, :, 3], op=ALU.add
            )
            nc.gpsimd.memset(shd[:], 0.0)
            nc.gpsimd.memset(shu[:], 0.0)
            for ct in range(CT):
                nc.gpsimd.dma_start(
                    out=shd[1:128, ct : ct + 1], in_=pool2[0:127, ct : ct + 1]
                )
                nc.gpsimd.dma_start(
                    out=shu[0:127, ct : ct + 1], in_=pool2[1:128, ct : ct + 1]
                )
            nc.gpsimd.dma_start(out=shd[0:1, 1:2], in_=pool2[127:128, 0:1])
            nc.gpsimd.dma_start(out=shu[127:128, 0:1], in_=pool2[0:1, 1:2])
            nc.vector.tensor_scalar(
                out=eca1, in0=shd[:], scalar1=wecab[:, 0:1], scalar2=None,
                op0=ALU.mult,
            )
            nc.vector.scalar_tensor_tensor(
                out=eca2, in0=pool2[:], scalar=wecab[:, 1:2], in1=eca1[:],
                op0=ALU.mult, op1=ALU.add,
            )
            nc.vector.scalar_tensor_tensor(
                out=eca1, in0=shu[:], scalar=wecab[:, 2:3], in1=eca2[:],
                op0=ALU.mult, op1=ALU.add,
            )

        # ---- main loop ---------------------------------------------------
        fused_t = [None] * NCHUNKS
        ypt_t = [None] * NCHUNKS
        pr_t = {}
        pending_yev = [None]  # (cj) whose mt2=1 yev/bn runs early next chunk

        def emit_mm1_tap(ci, k):
            """mm1 for tap k (both ct out-tiles) + its evict/product."""
            dve_stt = k in DVE_STT_TAPS
            pr = prod_pool.tile([128, CT, CHUNK], BF16, tag=f"pr{k}",
                                name=f"pr{k}")
            fsb = None
            if not dve_stt:
                fsb = fsb_pool.tile([128, CT, CHUNK], BF16, tag="fsb",
                                    name="fsb")
            for ct in range(CT):
                mt = k * CT + ct
                fp = fps.tile([128, CHUNK], F32, tag="fp", name="fp")
                for kt in range(CT):
                    lhsT = wf_sb[kt][:, mt * 128 : (mt + 1) * 128]
                    for nh in range(NH):
                        nc.tensor.matmul(
                            fp[:, nh * 512 : (nh + 1) * 512],
                            lhsT,
                            center(ci, kt, nh),
                            start=(kt == 0),
                            stop=(kt == CT - 1),
                        )
                if dve_stt:
                    nc.vector.scalar_tensor_tensor(
                        out=pr[:, ct, :].rearrange("p (r w) -> p r w", w=W),
                        in0=fp[:].rearrange("p (r w) -> p r w", w=W),
                        scalar=bfp_sb[:, mt : mt + 1],
                        in1=win_ct(ci, k, ct),
                        op0=ALU.add, op1=ALU.mult,
                    )
                else:
                    nc.scalar.activation(
                        out=fsb[:, ct, :], in_=fp[:], func=AF.Identity,
                        bias=bfp_sb[:, mt : mt + 1], scale=1.0,
                    )
            pr_t[k] = pr
            return fsb, pr

        def emit_prod(ci, k, fsb, pr):
            if k in POOL_TAPS:
                for ct in range(CT):
                    nc.gpsimd.tensor_tensor(
                        out=pr[:, ct, :].rearrange("p (r w) -> p r w", w=W),
                        in0=fsb[:, ct, :].rearrange("p (r w) -> p r w", w=W),
                        in1=win_ct(ci, k, ct),
                        op=ALU.mult,
                    )
            else:
                nc.vector.tensor_tensor(
                    out=pr[:].rearrange("p c (r w) -> p c r w", w=W),
                    in0=fsb[:].rearrange("p c (r w) -> p c r w", w=W),
                    in1=win_all(ci, k),
                    op=ALU.mult,
                )

        def addp(a, b):
            nc.vector.tensor_add(pr_t[a][:], pr_t[a][:], pr_t[b][:])

        def emit_mm2_part(ci, mt2, nh, xc_first=False):
            # xc_first: x-branch MMs first (they don't need the fused tile;
            # used for the last chunk where fused lands late)
            yp = yps.tile([128, 512], F32, tag="yp", name="yp")
            srcs = [0, 1] if not xc_first else [1, 0]
            for si, s in enumerate(srcs):
                for kt in range(CT):
                    if s == 0:
                        lhsT = wp_sb[kt][:, mt2 * 128 : (mt2 + 1) * 128]
                        rhs = fused_t[ci][:, kt, nh * 512 : (nh + 1) * 512]
                    else:
                        lhsT = weff[kt][:, mt2 * 128 : (mt2 + 1) * 128]
                        rhs = center(ci, kt, nh)
                    nc.tensor.matmul(
                        yp[:], lhsT, rhs,
                        start=(si == 0 and kt == 0),
                        stop=(si == 1 and kt == CT - 1),
                    )
            if ypt_t[ci] is None:
                ypt_t[ci] = [[None] * NH for _ in range(CT)]
            ypt_t[ci][mt2][nh] = yp

        def emit_yev(ci, mt2):
            r0 = ci * ROWS
            for nh in range(NH):
                src = ypt_t[ci][mt2][nh]
                dst = y_sb[mt2][:, r0 * W + nh * 512 : r0 * W + (nh + 1) * 512]
                nc.scalar.activation(out=dst, in_=src[:], func=AF.Copy)

        def emit_bn(ci, mt2):
            # stats from the freshly evicted bf16 y slices (frees PSUM
            # sooner than reading the mm2 PSUM tiles; bn_stats caps at 512)
            r0 = ci * ROWS
            for nh in range(NH):
                lo = r0 * W + nh * 512
                nc.vector.bn_stats(
                    out=stats_sb[mt2][:, ci * NH + nh, :],
                    in_=y_sb[mt2][:, lo : lo + 512],
                )

        def emit_weff():
            for kt in range(CT):
                nc.vector.tensor_scalar(
                    out=weff[kt][:], in0=wp_sb[kt][:],
                    scalar1=attn[:, kt : kt + 1], scalar2=None, op0=ALU.mult,
                )

        def emit_chunk(ci):
            cj = ci - 1
            first = ci == 0
            ft = prod_pool.tile([128, CT, CHUNK], BF16, tag="fused",
                                name="fused", bufs=2)
            for k in range(KS * KS):
                fsb, pr = emit_mm1_tap(ci, k)
                # hooks between mm1 and the DVE product.  mm2 of mt2=1 is
                # emitted at the END of this chunk (below) so the PE has
                # ready work to chew at the chunk boundary while the
                # scalar drain backlog clears.
                if k == 1 and pending_yev[0] is not None:
                    emit_yev(pending_yev[0], 1)
                    emit_bn(pending_yev[0], 1)
                    pending_yev[0] = None
                if not first:
                    if k == 6:
                        emit_mm2_part(cj, 0, 0)
                    elif k == 7:
                        emit_mm2_part(cj, 0, 1)
                    elif k == 8:
                        emit_yev(cj, 0)
                        emit_bn(cj, 0)
                else:
                    if k == 5:
                        pool_scalar(2)
                    elif k == 7:
                        pool_dve(3)
                if fsb is not None:
                    emit_prod(ci, k, fsb, pr)
                # add tree woven between taps (in-place into pr tiles)
                if k == 1:
                    addp(0, 1)
                elif k == 3:
                    addp(2, 3)
                    addp(0, 2)
                elif k == 5:
                    addp(4, 5)
                elif k == 7:
                    addp(6, 7)
                    addp(4, 6)
                    addp(0, 4)
                elif k == 8:
                    nc.vector.tensor_add(ft[:], pr_t[0][:], pr_t[8][:])
            fused_t[ci] = ft
            if not first:
                emit_mm2_part(cj, 1, 0)
                emit_mm2_part(cj, 1, 1)
                pending_yev[0] = cj
            else:
                emit_eca_combine()
                # sigmoid = 1/(1+exp(-x)) with Exp on scalar (same act table)
                nc.scalar.activation(out=eca2[:], in_=eca1[:], func=AF.Exp,
                                     bias=zb[:, 0:1], scale=-1.0)
                nc.vector.tensor_scalar(
                    out=attn, in0=eca2[:], scalar1=1.0, scalar2=None,
                    op0=ALU.add,
                )
                nc.vector.reciprocal(out=attn[:], in_=attn[:])
                emit_weff()

        pool_dve(0)
        pool_dve(1)
        for ci in range(NCHUNKS):
            emit_chunk(ci)
            if ci == 0:
                # collective warmup: emitted after chunk 0 so its barrier
                # trigger doesn't stall the tensor queue during the head
                nc.sync.dma_start(out=warm_in[:], in_=zb[:])
                nc.gpsimd.collective_compute(
                    "AllReduce", ALU.add, replica_groups=RG,
                    ins=[warm_in[:].opt()], outs=[warm_out[:].opt()],
                )

        # last chunk's mm2 + stats; aggregate each mt2 as soon as its
        # stats are complete so only the pack+dma remain after mm2(3,1,*)
        ps = consts.tile([128, CT, 2], F32, tag="ps", name="ps")

        def emit_aggr(mt2):
            mv = consts.tile([128, 2], F32, tag=f"mv{mt2}", name=f"mv{mt2}")
            nc.vector.bn_aggr(out=mv[:], in_=stats_sb[mt2][:])
            nc.vector.tensor_scalar(
                out=ps[:, mt2, 0:1], in0=mv[:, 0:1], scalar1=float(HW),
                scalar2=None, op0=ALU.mult,
            )
            nc.vector.scalar_tensor_tensor(
                out=ps[:, mt2, 1:2], in0=mv[:, 0:1], scalar=mv[:, 0:1],
                in1=mv[:, 1:2], op0=ALU.mult, op1=ALU.add,
            )
            nc.vector.tensor_scalar(
                out=ps[:, mt2, 1:2], in0=ps[:, mt2, 1:2], scalar1=float(HW),
                scalar2=None, op0=ALU.mult,
            )

        c3 = NCHUNKS - 1
        if pending_yev[0] is not None:
            emit_yev(pending_yev[0], 1)
            emit_bn(pending_yev[0], 1)
            pending_yev[0] = None
        for mt2 in range(CT):
            for nh in range(NH):
                emit_mm2_part(c3, mt2, nh, xc_first=True)
            # stats straight from the mm2 PSUM tiles (no wait on yev)
            r0 = c3 * ROWS
            for nh in range(NH):
                nc.vector.bn_stats(
                    out=stats_sb[mt2][:, c3 * NH + nh, :],
                    in_=ypt_t[c3][mt2][nh][:],
                )
            emit_aggr(mt2)
            emit_yev(c3, mt2)

        # exchange the per-channel sums in bf16 (halves the collective
        # payload; ~0.4% stats error, well inside the accuracy budget)
        psh = consts.tile([128, CT * 2], BF16, tag="psh", name="psh")
        nc.vector.tensor_scalar(
            out=psh[:], in0=ps.rearrange("p m two -> p (m two)"),
            scalar1=1.0, scalar2=None, op0=ALU.mult,
        )
        ps_b = dram.tile([128, CT * 2], BF16, tag="psb", name="psb")
        gs_b = dram.tile([128, CT * 2], BF16, tag="gsb", name="gsb",
                         addr_space="Shared")
        nc.sync.dma_start(out=ps_b[:], in_=psh[:])
        nc.gpsimd.collective_compute(
            "AllReduce", ALU.add, replica_groups=RG,
            ins=[ps_b[:].opt()], outs=[gs_b[:].opt()],
        )
        # preload the Sqrt act table while the collective runs; input is a
        # ps slice so the dependency-driven scheduler cannot hoist the
        # table switch ahead of the Identity/Copy evicts
        sqscr = consts.tile([128, 1], F32, tag="sqscr", name="sqscr")
        nc.scalar.activation(out=sqscr[:], in_=ps[:, 0, 0:1], func=AF.Sqrt,
                             bias=zb[:, 0:1], scale=1.0)
        gs = consts.tile([128, CT, 2], BF16, tag="gs", name="gs")
        nc.sync.dma_start(out=gs.rearrange("p m two -> p (m two)"), in_=gs_b[:])

        # ---- normalize and write out ------------------------------------
        minv = 1.0 / float(B * HW)
        mg = consts.tile([128, CT], F32, tag="mg", name="mg")
        vg = consts.tile([128, CT], F32, tag="vg", name="vg")
        rr = consts.tile([128, CT], F32, tag="rr", name="rr")
        tt = consts.tile([128, CT], F32, tag="tt", name="tt")
        ac = consts.tile([128, CT], F32, tag="ac", name="ac")
        bc = consts.tile([128, CT], F32, tag="bc", name="bc")
        nc.vector.tensor_scalar(
            out=mg[:], in0=gs[:, :, 0], scalar1=minv, scalar2=None, op0=ALU.mult
        )
        nc.vector.tensor_scalar(
            out=vg[:], in0=gs[:, :, 1], scalar1=minv, scalar2=None, op0=ALU.mult
        )
        nc.vector.tensor_tensor(out=tt[:], in0=mg[:], in1=mg[:], op=ALU.mult)
        nc.vector.tensor_tensor(out=vg[:], in0=vg[:], in1=tt[:], op=ALU.subtract)
        nc.vector.tensor_scalar(
            out=vg[:], in0=vg[:], scalar1=1.0, scalar2=BN_EPS,
            op0=ALU.mult, op1=ALU.add,
        )
        nc.scalar.activation(out=tt[:], in_=vg[:], func=AF.Sqrt,
                             bias=zb[:, 0:1], scale=1.0)
        nc.vector.reciprocal(out=rr[:], in_=tt[:])
        nc.vector.tensor_tensor(out=ac[:], in0=rr[:], in1=gam_sb[:], op=ALU.mult)
        nc.vector.tensor_tensor(out=bc[:], in0=mg[:], in1=ac[:], op=ALU.mult)
        nc.vector.tensor_tensor(out=bc[:], in0=bet_sb[:], in1=bc[:], op=ALU.subtract)

        NSL = 4
        SL = HW // NSL
        idx = 0
        for si in range(NSL):
            for mt2 in range(CT):
                sl = slice(si * SL, (si + 1) * SL)
                nc.vector.tensor_scalar(
                    out=y_sb[mt2][:, sl], in0=y_sb[mt2][:, sl],
                    scalar1=ac[:, mt2 : mt2 + 1], scalar2=bc[:, mt2 : mt2 + 1],
                    op0=ALU.mult, op1=ALU.add,
                )
                eng = nc.sync if idx % 2 == 0 else nc.scalar
                eng.dma_start(out=yout[mt2, :, sl], in_=y_sb[mt2][:, sl])
                idx += 1


_NC = None


def _build_nc(debug=False):
    nc = bacc.Bacc(
        "TRN2", target_bir_lowering=False, debug=debug, num_devices=NCORES
    )
    with tile.TileContext(nc, num_cores=NCORES) as tc:
        _emit(tc)
    nc.compile()
    return nc


def _get_nc():
    global _NC
    if _NC is None:
        _NC = _build_nc()
    return _NC


def _prep_in_maps(x, W_filter, b_filter, w_eca, W_proj, gamma, beta):
    bf = ml_dtypes.bfloat16
    x = np.asarray(x, np.float32)
    W_filter = np.asarray(W_filter, np.float32)
    b_filter = np.asarray(b_filter, np.float32)
    w_eca = np.asarray(w_eca, np.float32)
    W_proj = np.asarray(W_proj, np.float32)
    gamma = np.asarray(gamma, np.float32)
    beta = np.asarray(beta, np.float32)

    # gutter layout: row r at G + r*SW, col SW-1 of each row stays zero
    buf = np.zeros((B, C, XB), np.float32)
    xr = x.reshape(B, C, H, W)
    for r in range(H):
        buf[:, :, G + r * SW : G + r * SW + W] = xr[:, :, r, :]
    xg_h = np.ascontiguousarray(buf.reshape(B, CT, 128, XB)).astype(bf)

    # permute mm1 weights: o' = k*256 + c  (original o = c*9 + k)
    wperm = W_filter.reshape(C, KS * KS, C).transpose(1, 0, 2).reshape(KS * KS * C, C)
    wf_h = np.ascontiguousarray(wperm.T.reshape(CT, 128, MT1 * 128)).astype(bf)
    bperm = b_filter.reshape(C, KS * KS).T.reshape(KS * KS * C)

    wp_h = np.ascontiguousarray(
        (0.5 * W_proj).T.reshape(CT, 128, C).transpose(1, 0, 2).reshape(128, CT * C)
    ).astype(bf)
    misc_h = np.zeros((128, MT1 + 7), np.float32)
    misc_h[:, 0:MT1] = bperm.reshape(MT1, 128).T
    misc_h[:, MT1 : MT1 + 3] = (w_eca / float(HW)).reshape(1, 3)
    misc_h[:, MT1 + 3 : MT1 + 5] = gamma.reshape(CT, 128).T
    misc_h[:, MT1 + 5 : MT1 + 7] = beta.reshape(CT, 128).T

    in_maps = []
    for i in range(B):
        m = {
            "xg": xg_h[i],
            "wf": wf_h,
            "misc": misc_h,
            "wp": wp_h,
        }
        in_maps.append(m)
    return in_maps


last_result = None


def kernel(x, W_filter, b_filter, w_eca, W_proj, b_proj, gamma, beta):
    """Full-input, full-output DDF module on 8 NeuronCores."""
    global last_result
    # b_proj is mathematically cancelled by the batch-norm; unused.
    in_maps = _prep_in_maps(x, W_filter, b_filter, w_eca, W_proj, gamma, beta)
    nc = _get_nc()
    trace = bool(int(os.environ.get("DDF_TRACE", "0")))
    res = run_bass_kernel_spmd(nc, in_maps, list(range(NCORES)), trace=trace)
    last_result = res
    out = np.stack(
        [
            np.asarray(res.results[i]["y"]).reshape(C, H, W).astype(np.float32)
            for i in range(B)
        ]
    )
    return out


# revision 43
# speedup vs baseline: 1.1662x; 1.0301x over previous
"""Trainium2 Bass kernel for the DDF (dynamic-filter + ECA + BN) module.

Data-parallel over batch B=8 across 8 NeuronCores (one image per core),
params replicated, sync-BN via a single small AllReduce.

Layout: channels on partitions (CT=2 channel-tiles of 128); x lives in ONE
SBUF buffer with a 65-elem row stride whose 65th column is a zero "gutter".
All nine 3x3 window shifts are then plain slices of that buffer (the gutter
supplies the zero-pad at the row edges), so no derived shifted copies, no
wrap fix-ups.

Per 16-row chunk, the 18 mm1 PSUM tiles are drained by three engines in
parallel so the PE never waits on a PSUM bank:
  - Scalar taps: ACT evict (+bias) to SBUF, DVE does the bf16 window product.
  - Pool taps:   gpsimd scalar_tensor_tensor does (psum+bias)*window in one op.
The 9 tap products are summed by an 8-add DVE tree ordered by readiness, and
mm2 contracts {fused, attn*x} through W_proj (x-branch via attention-scaled
weights).  mm2 of chunk i is interleaved into mm1 of chunk i+1.  BN stats are
taken from the mm2 PSUM tiles (DVE bn_stats), aggregated, and exchanged with
one 2KB AllReduce on the sync queue; the Sqrt act-table load is hidden under
the AllReduce.  y is kept in bf16 and normalized in place, then written out
on two DMA queues.
"""

import os

import numpy as np
import ml_dtypes

import concourse.bass as bass
import concourse.mybir as mybir
import concourse.tile as tile
from concourse import bacc
from concourse.bass_utils import run_bass_kernel_spmd

B, C, H, W = 8, 256, 64, 64
KS = 3
HW = H * W                    # 4096
SW = W + 1                    # row stride with zero gutter column
G = SW + 1                    # guard elems at each end (covers di,dj = -1,-1)
XB = G + H * SW + G           # 4292 per channel-tile
NCORES = 8
CT = 2                        # channel tiles of 128
MT1 = KS * KS * CT            # 18 mm1 output m-tiles
BN_EPS = 1e-5
F32 = mybir.dt.float32
BF16 = mybir.dt.bfloat16
ROWS = 16                     # rows per chunk
NCHUNKS = H // ROWS           # 4
CHUNK = ROWS * W              # 1024 pixels per chunk per channel-tile
NH = CHUNK // 512             # 512-px matmul groups per chunk

AF = mybir.ActivationFunctionType
ALU = mybir.AluOpType
RG = [list(range(NCORES))]

# Pool cannot read PSUM on TRN2, and bulk gpsimd tensor_tensor traffic was
# measured to slow concurrent DVE ops ~2x (SBUF contention), so the drain
# pipeline uses Scalar (all 18 ACT evicts+bias) + DVE (all products, adds,
# bn) only — measured balanced at ~23us/chunk each.
POOL_TAPS = ()
DVE_STT_TAPS = (8,)           # drain tap 8 on DVE to shorten the chunk-end
                              # evict->product->fused critical chain


def _emit(tc):
    nc = tc.nc

    xgp = nc.declare_dram_parameter("xg", [CT, 128, XB], BF16, isOutput=False)
    wf = nc.declare_dram_parameter("wf", [CT, 128, MT1 * 128], BF16, isOutput=False)
    # misc fp32 params packed: bfp[18] | weca[3] | gam[2] | bet[2]
    misc = nc.declare_dram_parameter("misc", [128, MT1 + 7], F32, isOutput=False)
    wp = nc.declare_dram_parameter("wp", [128, CT * C], BF16, isOutput=False)
    yout = nc.declare_dram_parameter("y", [CT, 128, HW], BF16, isOutput=True)

    with (
        tc.tile_pool(name="consts", bufs=1) as consts,
        tc.tile_pool(name="fps", bufs=3, space="PSUM") as fps,
        tc.tile_pool(name="yps", bufs=2, space="PSUM") as yps,
        tc.tile_pool(name="fsb", bufs=5) as fsb_pool,
        tc.tile_pool(name="prod", bufs=1) as prod_pool,
        tc.tile_pool(name="dram", bufs=1, space="DRAM") as dram,
    ):
        # ---- resident tensors -------------------------------------------
        wf_sb = [consts.tile([128, MT1 * 128], BF16, tag=f"wf{kt}", name=f"wf{kt}")
                 for kt in range(CT)]
        wpb = consts.tile([128, CT, C], BF16, tag="wpb", name="wpb")
        wp_sb = [wpb[:, kt, :] for kt in range(CT)]
        weffb = consts.tile([128, CT, C], BF16, tag="weffb", name="weffb")
        weff = [weffb[:, kt, :] for kt in range(CT)]
        miscb = consts.tile([128, MT1 + 7], F32, tag="miscb", name="miscb")
        bfp_sb = miscb[:, 0:MT1]
        wecab = miscb[:, MT1 : MT1 + 3]
        gam_sb = miscb[:, MT1 + 3 : MT1 + 5]
        bet_sb = miscb[:, MT1 + 5 : MT1 + 7]
        xg = consts.tile([128, CT, XB], BF16, tag="xg", name="xg")
        y_sb = [consts.tile([128, HW], BF16, tag=f"ysb{mt}", name=f"ysb{mt}")
                for mt in range(CT)]
        stats_sb = [
            consts.tile([128, NCHUNKS * NH, 6], F32, tag=f"st{mt}", name=f"st{mt}")
            for mt in range(CT)
        ]
        pscr = consts.tile([128, ROWS * SW], F32, tag="pscr", name="pscr")
        pacc = consts.tile([128, CT, NCHUNKS], F32, tag="pacc", name="pacc")
        zb = consts.tile([128, 1], F32, tag="zb", name="zb")
        nc.vector.memset(zb[:], 0.0)

        warm_in = dram.tile([128, 1], F32, tag="wi", name="wi")
        warm_out = dram.tile([128, 1], F32, tag="wo", name="wo",
                             addr_space="Shared")

        # ---- input DMAs --------------------------------------------------
        # x in 4 overlapping row pieces per ct (halo rows included so chunk
        # i's windows only read pieces <= i); ct0 on sync, ct1 on gpsimd.
        # wf tiles split scalar/sync so both land within ~2.5us.
        pieces = []
        for i in range(NCHUNKS):
            lo = 0 if i == 0 else G + (ROWS * i - 1) * SW - 2
            hi = XB if i == NCHUNKS - 1 else G + (ROWS * i + ROWS + 1) * SW
            pieces.append((lo, hi))
        nc.sync.dma_start(out=xg[:, 0, pieces[0][0]:pieces[0][1]],
                          in_=xgp[0, :, pieces[0][0]:pieces[0][1]])
        nc.gpsimd.dma_start(out=xg[:, 1, pieces[0][0]:pieces[0][1]],
                            in_=xgp[1, :, pieces[0][0]:pieces[0][1]])
        nc.scalar.dma_start(out=wf_sb[0][:], in_=wf[0])
        nc.sync.dma_start(out=wf_sb[1][:], in_=wf[1])
        for lo, hi in pieces[1:]:
            nc.sync.dma_start(out=xg[:, 0, lo:hi], in_=xgp[0, :, lo:hi])
            nc.gpsimd.dma_start(out=xg[:, 1, lo:hi], in_=xgp[1, :, lo:hi])
        nc.scalar.dma_start(out=miscb[:], in_=misc[:, :])
        nc.scalar.dma_start(
            out=wpb.rearrange("p c x -> p (c x)"), in_=wp[:, :]
        )

        # ---- window / center access patterns ----------------------------
        def win_all(ci, k):
            di, dj = divmod(k, KS)
            off = G + (ROWS * ci + di - 1) * SW + (dj - 1)
            return xg[:, :, off : off + ROWS * SW].rearrange(
                "p c (r w) -> p c r w", w=SW)[:, :, :, 0:W]

        def win_ct(ci, k, ct):
            di, dj = divmod(k, KS)
            off = G + (ROWS * ci + di - 1) * SW + (dj - 1)
            return xg[:, ct, off : off + ROWS * SW].rearrange(
                "p (r w) -> p r w", w=SW)[:, :, 0:W]

        def center(ci, kt, nh):
            off = G + (ROWS * ci + 8 * nh) * SW
            return xg[:, kt, off : off + 8 * SW].rearrange(
                "p (r w) -> p r w", w=SW)[:, :, 0:W]

        # ---- ECA pooling -------------------------------------------------
        # pieces 0,1 on DVE (head slack), 2,3 on scalar accum (hooked)
        def pool_dve(ci):
            lo = G + ROWS * ci * SW
            nc.vector.tensor_reduce(
                out=pacc[:, :, ci : ci + 1],
                in_=xg[:, :, lo : lo + ROWS * SW],
                axis=mybir.AxisListType.X,
                op=ALU.add,
            )

        def pool_scalar(ci):
            lo = G + ROWS * ci * SW
            for ct in range(CT):
                nc.scalar.activation(
                    out=pscr[:], in_=xg[:, ct, lo : lo + ROWS * SW],
                    func=AF.Copy, accum_out=pacc[:, ct, ci : ci + 1],
                )



        pool2 = consts.tile([128, CT], F32, tag="pool2", name="pool2")
        shd = consts.tile([128, CT], F32, tag="shd", name="shd")
        shu = consts.tile([128, CT], F32, tag="shu", name="shu")
        eca1 = consts.tile([128, CT], F32, tag="eca1", name="eca1")
        eca2 = consts.tile([128, CT], F32, tag="eca2", name="eca2")
        attn = consts.tile([128, CT], F32, tag="attn", name="attn")

        def emit_eca_combine():
            nc.gpsimd.tensor_tensor(
                out=pool2[:], in0=pacc[:, :, 0], in1=pacc[:, :, 1], op=ALU.add
            )
            nc.gpsimd.tensor_tensor(
                out=pool2[:], in0=pool2[:], in1=pacc[:, :, 2], op=ALU.add
            )
            nc.gpsimd.tensor_tensor(
                out=pool2[:], in0=pool2[:], in1=pacc[:, :, 3], op=ALU.add
            )
            nc.gpsimd.memset(shd[:], 0.0)
            nc.gpsimd.memset(shu[:], 0.0)
            for ct in range(CT):
                nc.gpsimd.dma_start(
                    out=shd[1:128, ct : ct + 1], in_=pool2[0:127, ct : ct + 1]
                )
                nc.gpsimd.dma_start(
                    out=shu[0:127, ct : ct + 1], in_=pool2[1:128, ct : ct + 1]
                )
            nc.gpsimd.dma_start(out=shd[0:1, 1:2], in_=pool2[127:128, 0:1])
            nc.gpsimd.dma_start(out=shu[127:128, 0:1], in_=pool2[0:1, 1:2])
            nc.vector.tensor_scalar(
                out=eca1, in0=shd[:], scalar1=wecab[:, 0:1], scalar2=None,
                op0=ALU.mult,
            )
            nc.vector.scalar_tensor_tensor(
                out=eca2, in0=pool2[:], scalar=wecab[:, 1:2], in1=eca1[:],
                op0=ALU.mult, op1=ALU.add,
            )
            nc.vector.scalar_tensor_tensor(
                out=eca1, in0=shu[:], scalar=wecab[:, 2:3], in1=eca2[:],
                op0=ALU.mult, op1=ALU.add,
            )

        # ---- main loop ---------------------------------------------------
        fused_t = [None] * NCHUNKS
        ypt_t = [None] * NCHUNKS
        pr_t = {}
        pending_yev = [None]  # (cj) whose mt2=1 yev/bn runs early next chunk

        def emit_mm1_tap(ci, k):
            """mm1 for tap k (both ct out-tiles) + its evict/product."""
            dve_stt = k in DVE_STT_TAPS
            pr = prod_pool.tile([128, CT, CHUNK], BF16, tag=f"pr{k}",
                                name=f"pr{k}")
            fsb = None
            if not dve_stt:
                fsb = fsb_pool.tile([128, CT, CHUNK], BF16, tag="fsb",
                                    name="fsb")
            for ct in range(CT):
                mt = k * CT + ct
                fp = fps.tile([128, CHUNK], F32, tag="fp", name="fp")
                for kt in range(CT):
                    lhsT = wf_sb[kt][:, mt * 128 : (mt + 1) * 128]
                    for nh in range(NH):
                        nc.tensor.matmul(
                            fp[:, nh * 512 : (nh + 1) * 512],
                            lhsT,
                            center(ci, kt, nh),
                            start=(kt == 0),
                            stop=(kt == CT - 1),
                        )
                if dve_stt:
                    nc.vector.scalar_tensor_tensor(
                        out=pr[:, ct, :].rearrange("p (r w) -> p r w", w=W),
                        in0=fp[:].rearrange("p (r w) -> p r w", w=W),
                        scalar=bfp_sb[:, mt : mt + 1],
                        in1=win_ct(ci, k, ct),
                        op0=ALU.add, op1=ALU.mult,
                    )
                else:
                    nc.scalar.activation(
                        out=fsb[:, ct, :], in_=fp[:], func=AF.Identity,
                        bias=bfp_sb[:, mt : mt + 1], scale=1.0,
                    )
            pr_t[k] = pr
            return fsb, pr

        def emit_prod(ci, k, fsb, pr):
            if k in POOL_TAPS:
                for ct in range(CT):
                    nc.gpsimd.tensor_tensor(
                        out=pr[:, ct, :].rearrange("p (r w) -> p r w", w=W),
                        in0=fsb[:, ct, :].rearrange("p (r w) -> p r w", w=W),
                        in1=win_ct(ci, k, ct),
                        op=ALU.mult,
                    )
            else:
                nc.vector.tensor_tensor(
                    out=pr[:].rearrange("p c (r w) -> p c r w", w=W),
                    in0=fsb[:].rearrange("p c (r w) -> p c r w", w=W),
                    in1=win_all(ci, k),
                    op=ALU.mult,
                )

        def addp(a, b):
            nc.vector.tensor_add(pr_t[a][:], pr_t[a][:], pr_t[b][:])

        def emit_mm2_part(ci, mt2, nh, xc_first=False):
            # xc_first: x-branch MMs first (they don't need the fused tile;
            # used for the last chunk where fused lands late)
            yp = yps.tile([128, 512], F32, tag="yp", name="yp")
            srcs = [0, 1] if not xc_first else [1, 0]
            for si, s in enumerate(srcs):
                for kt in range(CT):
                    if s == 0:
                        lhsT = wp_sb[kt][:, mt2 * 128 : (mt2 + 1) * 128]
                        rhs = fused_t[ci][:, kt, nh * 512 : (nh + 1) * 512]
                    else:
                        lhsT = weff[kt][:, mt2 * 128 : (mt2 + 1) * 128]
                        rhs = center(ci, kt, nh)
                    nc.tensor.matmul(
                        yp[:], lhsT, rhs,
                        start=(si == 0 and kt == 0),
                        stop=(si == 1 and kt == CT - 1),
                    )
            if ypt_t[ci] is None:
                ypt_t[ci] = [[None] * NH for _ in range(CT)]
            ypt_t[ci][mt2][nh] = yp

        def emit_yev(ci, mt2):
            r0 = ci * ROWS
            for nh in range(NH):
                src = ypt_t[ci][mt2][nh]
                dst = y_sb[mt2][:, r0 * W + nh * 512 : r0 * W + (nh + 1) * 512]
                nc.scalar.activation(out=dst, in_=src[:], func=AF.Copy)

        def emit_bn(ci, mt2):
            # stats from the freshly evicted bf16 y slices (frees PSUM
            # sooner than reading the mm2 PSUM tiles; bn_stats caps at 512)
            r0 = ci * ROWS
            for nh in range(NH):
                lo = r0 * W + nh * 512
                nc.vector.bn_stats(
                    out=stats_sb[mt2][:, ci * NH + nh, :],
                    in_=y_sb[mt2][:, lo : lo + 512],
                )

        def emit_weff():
            for kt in range(CT):
                nc.vector.tensor_scalar(
                    out=weff[kt][:], in0=wp_sb[kt][:],
                    scalar1=attn[:, kt : kt + 1], scalar2=None, op0=ALU.mult,
                )

        def emit_chunk(ci):
            cj = ci - 1
            first = ci == 0
            ft = prod_pool.tile([128, CT, CHUNK], BF16, tag="fused",
                                name="fused", bufs=2)
            for k in range(KS * KS):
                fsb, pr = emit_mm1_tap(ci, k)
                # hooks between mm1 and the DVE product.  mm2 of mt2=1 is
                # emitted at the END of this chunk (below) so the PE has
                # ready work to chew at the chunk boundary while the
                # scalar drain backlog clears.
                if k == 1 and pending_yev[0] is not None:
                    emit_yev(pending_yev[0], 1)
                    emit_bn(pending_yev[0], 1)
                    pending_yev[0] = None
                if not first:
                    if k == 6:
                        emit_mm2_part(cj, 0, 0)
                    elif k == 7:
                        emit_mm2_part(cj, 0, 1)
                    elif k == 8:
                        emit_yev(cj, 0)
                        emit_bn(cj, 0)
                else:
                    if k == 5:
                        pool_scalar(2)
                    elif k == 7:
                        pool_dve(3)
                if fsb is not None:
                    emit_prod(ci, k, fsb, pr)
                # add tree woven between taps (in-place into pr tiles)
                if k == 1:
                    addp(0, 1)
                elif k == 3:
                    addp(2, 3)
                    addp(0, 2)
                elif k == 5:
                    addp(4, 5)
                elif k == 7:
                    addp(6, 7)
                    addp(4, 6)
                    addp(0, 4)
                elif k == 8:
                    nc.vector.tensor_add(ft[:], pr_t[0][:], pr_t[8][:])
            fused_t[ci] = ft
            if not first:
                emit_mm2_part(cj, 1, 0)
                emit_mm2_part(cj, 1, 1)
                pending_yev[0] = cj
            else:
                emit_eca_combine()
                # sigmoid = 1/(1+exp(-x)) with Exp on scalar (same act table)
                nc.scalar.activation(out=eca2[:], in_=eca1[:], func=AF.Exp,
                                     bias=zb[:, 0:1], scale=-1.0)
                nc.vector.tensor_scalar(
                    out=attn, in0=eca2[:], scalar1=1.0, scalar2=None,
                    op0=ALU.add,
                )
                nc.vector.reciprocal(out=attn[:], in_=attn[:])
                emit_weff()

        pool_dve(0)
        pool_dve(1)
        for ci in range(NCHUNKS):
            emit_chunk(ci)
            if ci == 0:
                # collective warmup.  warm_in sources attn (ready at the end
                # of chunk 0) so the dependency-driven scheduler cannot
                # hoist the barrier trigger into the head, where it stalls
                # the tensor queue before the first matmul.
                nc.sync.dma_start(out=warm_in[:], in_=attn[:, 0:1])
                nc.gpsimd.collective_compute(
                    "AllReduce", ALU.add, replica_groups=RG,
                    ins=[warm_in[:].opt()], outs=[warm_out[:].opt()],
                )

        # last chunk's mm2 + stats; aggregate each mt2 as soon as its
        # stats are complete so only the pack+dma remain after mm2(3,1,*)
        ps = consts.tile([128, CT, 2], F32, tag="ps", name="ps")

        def emit_aggr(mt2):
            mv = consts.tile([128, 2], F32, tag=f"mv{mt2}", name=f"mv{mt2}")
            nc.vector.bn_aggr(out=mv[:], in_=stats_sb[mt2][:])
            nc.vector.tensor_scalar(
                out=ps[:, mt2, 0:1], in0=mv[:, 0:1], scalar1=float(HW),
                scalar2=None, op0=ALU.mult,
            )
            nc.vector.scalar_tensor_tensor(
                out=ps[:, mt2, 1:2], in0=mv[:, 0:1], scalar=mv[:, 0:1],
                in1=mv[:, 1:2], op0=ALU.mult, op1=ALU.add,
            )
            nc.vector.tensor_scalar(
                out=ps[:, mt2, 1:2], in0=ps[:, mt2, 1:2], scalar1=float(HW),
                scalar2=None, op0=ALU.mult,
            )

        c3 = NCHUNKS - 1
        if pending_yev[0] is not None:
            emit_yev(pending_yev[0], 1)
            emit_bn(pending_yev[0], 1)
            pending_yev[0] = None
        for mt2 in range(CT):
            for nh in range(NH):
                emit_mm2_part(c3, mt2, nh, xc_first=True)
            # stats straight from the mm2 PSUM tiles (no wait on yev)
            r0 = c3 * ROWS
            for nh in range(NH):
                nc.vector.bn_stats(
                    out=stats_sb[mt2][:, c3 * NH + nh, :],
                    in_=ypt_t[c3][mt2][nh][:],
                )
            emit_aggr(mt2)
            emit_yev(c3, mt2)

        # exchange the per-channel sums in bf16 (halves the collective
        # payload; ~0.4% stats error, well inside the accuracy budget)
        psh = consts.tile([128, CT * 2], BF16, tag="psh", name="psh")
        nc.vector.tensor_scalar(
            out=psh[:], in0=ps.rearrange("p m two -> p (m two)"),
            scalar1=1.0, scalar2=None, op0=ALU.mult,
        )
        ps_b = dram.tile([128, CT * 2], BF16, tag="psb", name="psb")
        gs_b = dram.tile([128, CT * 2], BF16, tag="gsb", name="gsb",
                         addr_space="Shared")
        nc.sync.dma_start(out=ps_b[:], in_=psh[:])
        nc.gpsimd.collective_compute(
            "AllReduce", ALU.add, replica_groups=RG,
            ins=[ps_b[:].opt()], outs=[gs_b[:].opt()],
        )
        # preload the Sqrt act table while the collective runs; input is a
        # ps slice so the dependency-driven scheduler cannot hoist the
        # table switch ahead of the Identity/Copy evicts
        sqscr = consts.tile([128, 1], F32, tag="sqscr", name="sqscr")
        nc.scalar.activation(out=sqscr[:], in_=ps[:, 0, 0:1], func=AF.Sqrt,
                             bias=zb[:, 0:1], scale=1.0)
        gs = consts.tile([128, CT, 2], BF16, tag="gs", name="gs")
        nc.sync.dma_start(out=gs.rearrange("p m two -> p (m two)"), in_=gs_b[:])

        # ---- normalize and write out ------------------------------------
        minv = 1.0 / float(B * HW)
        mg = consts.tile([128, CT], F32, tag="mg", name="mg")
        vg = consts.tile([128, CT], F32, tag="vg", name="vg")
        rr = consts.tile([128, CT], F32, tag="rr", name="rr")
        tt = consts.tile([128, CT], F32, tag="tt", name="tt")
        ac = consts.tile([128, CT], F32, tag="ac", name="ac")
        bc = consts.tile([128, CT], F32, tag="bc", name="bc")
        nc.vector.tensor_scalar(
            out=mg[:], in0=gs[:, :, 0], scalar1=minv, scalar2=None, op0=ALU.mult
        )
        nc.vector.tensor_scalar(
            out=vg[:], in0=gs[:, :, 1], scalar1=minv, scalar2=None, op0=ALU.mult
        )
        nc.vector.tensor_tensor(out=tt[:], in0=mg[:], in1=mg[:], op=ALU.mult)
        nc.vector.tensor_tensor(out=vg[:], in0=vg[:], in1=tt[:], op=ALU.subtract)
        nc.vector.tensor_scalar(
            out=vg[:], in0=vg[:], scalar1=1.0, scalar2=BN_EPS,
            op0=ALU.mult, op1=ALU.add,
        )
        nc.scalar.activation(out=tt[:], in_=vg[:], func=AF.Sqrt,
                             bias=zb[:, 0:1], scale=1.0)
        nc.vector.reciprocal(out=rr[:], in_=tt[:])
        nc.vector.tensor_tensor(out=ac[:], in0=rr[:], in1=gam_sb[:], op=ALU.mult)
        nc.vector.tensor_tensor(out=bc[:], in0=mg[:], in1=ac[:], op=ALU.mult)
        nc.vector.tensor_tensor(out=bc[:], in0=bet_sb[:], in1=bc[:], op=ALU.subtract)

        NSL = 4
        SL = HW // NSL
        idx = 0
        for si in range(NSL):
            for mt2 in range(CT):
                sl = slice(si * SL, (si + 1) * SL)
                nc.vector.tensor_scalar(
                    out=y_sb[mt2][:, sl], in0=y_sb[mt2][:, sl],
                    scalar1=ac[:, mt2 : mt2 + 1], scalar2=bc[:, mt2 : mt2 + 1],
                    op0=ALU.mult, op1=ALU.add,
                )
                eng = nc.sync if idx % 2 == 0 else nc.scalar
                eng.dma_start(out=yout[mt2, :, sl], in_=y_sb[mt2][:, sl])
                idx += 1


_NC = None


def _build_nc(debug=False):
    nc = bacc.Bacc(
        "TRN2", target_bir_lowering=False, debug=debug, num_devices=NCORES
    )
    with tile.TileContext(nc, num_cores=NCORES) as tc:
        _emit(tc)
    nc.compile()
    return nc


def _get_nc():
    global _NC
    if _NC is None:
        _NC = _build_nc()
    return _NC


def _prep_in_maps(x, W_filter, b_filter, w_eca, W_proj, gamma, beta):
    bf = ml_dtypes.bfloat16
    x = np.asarray(x, np.float32)
    W_filter = np.asarray(W_filter, np.float32)
    b_filter = np.asarray(b_filter, np.float32)
    w_eca = np.asarray(w_eca, np.float32)
    W_proj = np.asarray(W_proj, np.float32)
    gamma = np.asarray(gamma, np.float32)
    beta = np.asarray(beta, np.float32)

    # gutter layout: row r at G + r*SW, col SW-1 of each row stays zero
    buf = np.zeros((B, C, XB), np.float32)
    xr = x.reshape(B, C, H, W)
    for r in range(H):
        buf[:, :, G + r * SW : G + r * SW + W] = xr[:, :, r, :]
    xg_h = np.ascontiguousarray(buf.reshape(B, CT, 128, XB)).astype(bf)

    # permute mm1 weights: o' = k*256 + c  (original o = c*9 + k)
    wperm = W_filter.reshape(C, KS * KS, C).transpose(1, 0, 2).reshape(KS * KS * C, C)
    wf_h = np.ascontiguousarray(wperm.T.reshape(CT, 128, MT1 * 128)).astype(bf)
    bperm = b_filter.reshape(C, KS * KS).T.reshape(KS * KS * C)

    wp_h = np.ascontiguousarray(
        (0.5 * W_proj).T.reshape(CT, 128, C).transpose(1, 0, 2).reshape(128, CT * C)
    ).astype(bf)
    misc_h = np.zeros((128, MT1 + 7), np.float32)
    misc_h[:, 0:MT1] = bperm.reshape(MT1, 128).T
    misc_h[:, MT1 : MT1 + 3] = (w_eca / float(HW)).reshape(1, 3)
    misc_h[:, MT1 + 3 : MT1 + 5] = gamma.reshape(CT, 128).T
    misc_h[:, MT1 + 5 : MT1 + 7] = beta.reshape(CT, 128).T

    in_maps = []
    for i in range(B):
        m = {
            "xg": xg_h[i],
            "wf": wf_h,
            "misc": misc_h,
            "wp": wp_h,
        }
        in_maps.append(m)
    return in_maps


last_result = None


def kernel(x, W_filter, b_filter, w_eca, W_proj, b_proj, gamma, beta):
    """Full-input, full-output DDF module on 8 NeuronCores."""
    global last_result
    # b_proj is mathematically cancelled by the batch-norm; unused.
    in_maps = _prep_in_maps(x, W_filter, b_filter, w_eca, W_proj, gamma, beta)
    nc = _get_nc()
    trace = bool(int(os.environ.get("DDF_TRACE", "0")))
    res = run_bass_kernel_spmd(nc, in_maps, list(range(NCORES)), trace=trace)
    last_result = res
    out = np.stack(
        [
            np.asarray(res.results[i]["y"]).reshape(C, H, W).astype(np.float32)
            for i in range(B)
        ]
    )
    return out
